# revision 1
# baseline (speedup 1.0000x reference)
"""Trainium2 Bass kernel for nn_ClassicalQuantumAttention.

Data-parallel over batch: 128 batch elems -> 16 per NeuronCore x 8 cores.

Per-core pipeline (per batch elem b):
  scores path : hpreT[j,T] = [Wfold;bfold]^T @ [x;1]  (weights folded on host)
                tanh (ACT) -> scoresT[1,T] = att_w2^T @ tanhT  (PE)
                DMA-scatter scores -> [nc, 16] layout, softmax on DVE/ACT
  chunk path  : xw[nc,C] = sum_k w[nc,k] * xperm[nc, k, :]   (STT chain)
                xwT = PE-transpose;  chunksT = [emb_w;emb_b]^T @ [xwT;1]
                params = sigmoid(chunks @ proj_w + proj_b)   (PE + ACT)
  quantum     : 6-qubit statevector per (b, chunk): 128 states on partitions,
                state = [64 re | 64 im] on free dim.  Gates via fused
                scalar_tensor_tensor with per-partition cos/sin scalars.
  LCU         : mixed = sum_nc coeffs[nc] * evolved  (PE, coeffs from host)
  tail        : normalize, qff ansatz (shared params), expvals (TTR),
                out head + layernorm + classifier (PE + small ops)
"""

import numpy as np
import sys

for _p in ("/opt/trn_rl_repo",):
    if _p not in sys.path:
        sys.path.insert(0, _p)

import concourse.bass as bass
import concourse.tile as tile
from concourse import mybir
from concourse.bass_utils import run_bass_kernel_spmd

F32 = mybir.dt.float32
ALU = mybir.AluOpType
AF = mybir.ActivationFunctionType
AX = mybir.AxisListType

N_CORES = 8
B_TOT = 128
BPC = B_TOT // N_CORES  # 16 batch elems per core
C_IN = 64
T = 2048
D = 256
CH = 16
NC = T // CH  # 128 chunks
NQ = 6
DIM = 64  # 2**6 amplitudes
STF = 2 * DIM  # 128 floats per state ([64 re | 64 im])


# ---------------------------------------------------------------- gate list
def ansatz_gates(n_layers):
    """[(kind, wire-or-(ctrl,tgt), param_idx)] matching reference _ansatz."""
    gates = []
    idx = 0
    for _ in range(n_layers):
        for i in range(NQ):
            gates.append(("rx", i, idx))
            gates.append(("ry", i, idx + 1))
            gates.append(("rz", i, idx + 2))
            idx += 3
        for i in range(NQ):
            gates.append(("crx", (i, (i + 1) % NQ), idx))
            idx += 1
        for i in range(NQ - 1, -1, -1):
            gates.append(("crx", (i, (i - 1) % NQ), idx))
            idx += 1
    return gates


# ------------------------------------------------------------- AP helpers
def amp_view(t, ri, fixed, swap_p=None, split_ps=()):
    """Strided view of a statevector AP t ([P, 128] = [P, (ri, amp6bits)]).

    ri: 0 (re), 1 (im), or None (both -> extra leading free dim).
    fixed: {bit_pos: 0/1} fixes amplitude bits (wire w <-> bit 5-w).
    swap_p: bit position iterated in order (1, 0) via negative step.
    split_ps: bit positions forced into their own [step, 2] dim (to shape-match
              a swap view on another tensor).
    """
    part = t.ap[0]  # partition dim
    offset = t.offset
    dims = []
    if ri is None:
        dims.append([DIM, 2])
    else:
        offset += ri * DIM
    run = None  # [step, count]
    for p in range(5, -1, -1):
        if p in fixed:
            if run is not None:
                dims.append(run)
                run = None
            offset += fixed[p] << p
        elif swap_p == p:
            if run is not None:
                dims.append(run)
                run = None
            dims.append([-(1 << p), 2])
            offset += 1 << p
        elif p in split_ps:
            if run is not None:
                dims.append(run)
                run = None
            dims.append([1 << p, 2])
        else:
            if run is None:
                run = [1 << p, 2]
            else:
                run = [1 << p, run[1] * 2]
    if run is not None:
        dims.append(run)
    if not dims:
        dims.append([1, 1])
    # walrus compute ops accept at most 3 total dims (partition + 2 free)
    assert len(dims) <= 2, f"too many free dims: {dims}"
    return bass.AP(tensor=t.tensor, offset=offset, ap=[list(part)] + dims)


# ------------------------------------------------------------ gate emitters
def g_rx_first(eng, st, c, s, ns, pq, sup):
    """RX on bit pq when all bits <= pq are zero (sparse start)."""
    eng.tensor_scalar_mul(
        amp_view(st, 0, {**sup, pq: 1}), amp_view(st, 1, {**sup, pq: 0}), s
    )
    eng.tensor_scalar_mul(
        amp_view(st, 1, {**sup, pq: 1}), amp_view(st, 0, {**sup, pq: 0}), ns
    )
    v0 = amp_view(st, None, {**sup, pq: 0})
    eng.tensor_scalar_mul(v0, v0, c)


def g_rx(eng, st, B, c, s, pq):
    eng.tensor_scalar_mul(B, st, s)
    for k in (0, 1):
        o = amp_view(st, 0, {pq: k})
        eng.scalar_tensor_tensor(
            o, o, c, amp_view(B, 1, {pq: 1 - k}), ALU.mult, ALU.add
        )
        o = amp_view(st, 1, {pq: k})
        eng.scalar_tensor_tensor(
            o, o, c, amp_view(B, 0, {pq: 1 - k}), ALU.mult, ALU.subtract
        )


def g_ry(eng, st, B, c, s, pq, sup):
    eng.tensor_scalar_mul(amp_view(B, None, sup), amp_view(st, None, sup), s)
    for ri in (0, 1):
        o = amp_view(st, ri, {**sup, pq: 0})
        eng.scalar_tensor_tensor(
            o, o, c, amp_view(B, ri, {**sup, pq: 1}), ALU.mult, ALU.subtract
        )
    for ri in (0, 1):
        o = amp_view(st, ri, {**sup, pq: 1})
        eng.scalar_tensor_tensor(
            o, o, c, amp_view(B, ri, {**sup, pq: 0}), ALU.mult, ALU.add
        )


def g_rz(eng, st, B, c, s, pq, sup):
    eng.tensor_scalar_mul(amp_view(B, None, sup), amp_view(st, None, sup), s)
    for ri, k, op in (
        (0, 0, ALU.add),
        (1, 0, ALU.subtract),
        (0, 1, ALU.subtract),
        (1, 1, ALU.add),
    ):
        o = amp_view(st, ri, {**sup, pq: k})
        eng.scalar_tensor_tensor(
            o, o, c, amp_view(B, 1 - ri, {**sup, pq: k}), ALU.mult, op
        )


def g_crx(eng, st, B, c, s, pc, pt):
    for ri in (0, 1):
        eng.tensor_scalar_mul(
            amp_view(B, ri, {pc: 1}), amp_view(st, ri, {pc: 1}), s
        )
    for kt in (0, 1):
        o = amp_view(st, 0, {pc: 1, pt: kt})
        eng.scalar_tensor_tensor(
            o, o, c, amp_view(B, 1, {pc: 1, pt: 1 - kt}), ALU.mult, ALU.add
        )
        o = amp_view(st, 1, {pc: 1, pt: kt})
        eng.scalar_tensor_tensor(
            o, o, c, amp_view(B, 0, {pc: 1, pt: 1 - kt}), ALU.mult, ALU.subtract
        )


def emit_ansatz(eng, st, B, col, n_layers, sparse):
    """col(j, kind) -> [P,1] AP of cos/sin/negsin for param j."""
    gates = ansatz_gates(n_layers)
    for gi, (kind, loc, j) in enumerate(gates):
        c = col(j, "c")
        s = col(j, "s")
        if kind == "crx":
            wc, wt = loc
            g_crx(eng, st, B, c, s, 5 - wc, 5 - wt)
        else:
            pq = 5 - loc
            in_l0 = sparse and gi < 3 * NQ
            sup = {p: 0 for p in range(pq)} if in_l0 else {}
            if kind == "rx":
                if in_l0:
                    g_rx_first(eng, st, c, s, col(j, "n"), pq, sup)
                else:
                    g_rx(eng, st, B, c, s, pq)
            elif kind == "ry":
                g_ry(eng, st, B, c, s, pq, sup)
            else:
                g_rz(eng, st, B, c, s, pq, sup)


def _split_multi_waits(nc):
    """This walrus build allows at most ONE sync-wait per instruction.

    Hoist extra waits onto same-engine NoOps inserted immediately before the
    offending instruction (engine stalls on the nops first - semantically
    identical).
    """
    ctr = [0]
    for f in nc.m.functions:
        for b in f.blocks:
            new = []
            for inst in b.instructions:
                si = inst.sync_info
                if si is not None and len(si.on_wait) > 1:
                    waits = list(si.on_wait)
                    for w in waits[:-1]:
                        ctr[0] += 1
                        nop = mybir.InstNoOp(
                            name=f"wsplit-{ctr[0]}",
                            ins=[],
                            outs=[],
                            engine=inst.engine,
                            sync_info=mybir.SyncInfo(on_wait=[w], on_update=[]),
                        )
                        new.append(nop)
                    inst.sync_info = mybir.SyncInfo(
                        on_wait=[waits[-1]], on_update=list(si.on_update)
                    )
                new.append(inst)
            b.instructions = new


# ---------------------------------------------------------------- program
def build_program(split_waits=True):
    nc = bass.Bass()

    # register extra activation-bias constants (pi/2 for cos-via-sin, ln eps)
    for v in (float(np.pi / 2), 1e-5):
        t = nc.alloc_sbuf_tensor(f"const-f32-{v}", [128, 1], F32)
        nc.gpsimd.memset(t.ap(), v)
        nc.const_aps.aps[(F32, v)] = t.ap()
    nc.all_engine_barrier()

    # ---- dram I/O (per core) ----
    xs = nc.declare_dram_parameter("xs", [BPC, C_IN, T], F32, isOutput=False)
    xp = nc.declare_dram_parameter("xp", [BPC, NC, CH * C_IN], F32, isOutput=False)
    wfb = nc.declare_dram_parameter("wfb", [C_IN + 1, 128], F32, isOutput=False)
    aw2 = nc.declare_dram_parameter("aw2", [128, 1], F32, isOutput=False)
    ewb = nc.declare_dram_parameter("ewb", [C_IN + 1, D], F32, isOutput=False)
    pjw = nc.declare_dram_parameter("pjw", [128, 120], F32, isOutput=False)
    pjb = nc.declare_dram_parameter("pjb", [1, 60], F32, isOutput=False)
    cf2 = nc.declare_dram_parameter("cf2", [NC, 2], F32, isOutput=False)
    qfc = nc.declare_dram_parameter("qfc", [BPC, 30], F32, isOutput=False)
    qfs = nc.declare_dram_parameter("qfs", [BPC, 30], F32, isOutput=False)
    qfn = nc.declare_dram_parameter("qfn", [BPC, 30], F32, isOutput=False)
    owb = nc.declare_dram_parameter("owb", [19, D], F32, isOutput=False)
    lng = nc.declare_dram_parameter("lng", [BPC, D], F32, isOutput=False)
    lnb = nc.declare_dram_parameter("lnb", [BPC, D], F32, isOutput=False)
    cw1 = nc.declare_dram_parameter("cw1", [128, 2 * D], F32, isOutput=False)
    cb1 = nc.declare_dram_parameter("cb1", [1, D], F32, isOutput=False)
    cw2 = nc.declare_dram_parameter("cw2", [128, 4], F32, isOutput=False)
    cb2 = nc.declare_dram_parameter("cb2", [1, 2], F32, isOutput=False)
    idn = nc.declare_dram_parameter("idn", [128, 128], F32, isOutput=False)
    out = nc.declare_dram_parameter("out", [BPC, 2], F32, isOutput=True)

    with tile.TileContext(nc) as tc:
        with (
            tc.tile_pool(name="const", bufs=1) as cp,
            tc.tile_pool(name="xbuf", bufs=2) as xpool,
            tc.tile_pool(name="xpbuf", bufs=2) as xppool,
            tc.tile_pool(name="tanh", bufs=2) as thpool,
            tc.tile_pool(name="perb", bufs=1) as pb,
            tc.tile_pool(name="stp", bufs=1) as stp,
            tc.tile_pool(name="small", bufs=4) as sm,
            tc.tile_pool(name="ps_h", bufs=2, space="PSUM") as ps_h,
            tc.tile_pool(name="ps_s", bufs=2, space="PSUM") as ps_s,
            tc.tile_pool(name="ps_m", bufs=2, space="PSUM") as ps_m,
            tc.tile_pool(name="ps_t", bufs=2, space="PSUM") as ps_t,
        ):
            # ---------------- constants into SBUF ----------------
            def cload(name, dram, shape):
                t = cp.tile(shape, F32, tag=name, name=name)
                nc.sync.dma_start(out=t, in_=dram[:, :])
                return t

            wfb_s = cload("wfb", wfb, [C_IN + 1, 128])
            aw2_s = cload("aw2", aw2, [128, 1])
            ewb_s = cload("ewb", ewb, [C_IN + 1, D])
            pjw_s = cload("pjw", pjw, [128, 120])
            pjb_s = cload("pjb", pjb, [1, 60])
            cf2_s = cload("cf2", cf2, [NC, 2])
            qfc_s = cload("qfc", qfc, [BPC, 30])
            qfs_s = cload("qfs", qfs, [BPC, 30])
            qfn_s = cload("qfn", qfn, [BPC, 30])
            owb_s = cload("owb", owb, [19, D])
            lng_s = cload("lng", lng, [BPC, D])
            lnb_s = cload("lnb", lnb, [BPC, D])
            cw1_s = cload("cw1", cw1, [128, 2 * D])
            cb1_s = cload("cb1", cb1, [1, D])
            cw2_s = cload("cw2", cw2, [128, 4])
            cb2_s = cload("cb2", cb2, [1, 2])
            idn_s = cload("idn", idn, [128, 128])

            ones = cp.tile([1, 128], F32, tag="ones")
            nc.vector.memset(ones, 1.0)

            # persistent per-group score tiles
            sc_g = [cp.tile([NC, 8 * CH], F32, tag=f"scg{g}", name=f"scg{g}") for g in range(2)]
            esc_g = [cp.tile([NC, 8 * CH], F32, tag=f"escg{g}", name=f"escg{g}") for g in range(2)]
            w_g = [cp.tile([NC, 8 * CH], F32, tag=f"wg{g}", name=f"wg{g}") for g in range(2)]

            B_dve = cp.tile([128, STF], F32, tag="Bdve")
            B_dve2 = cp.tile([128, STF], F32, tag="Bdve2")

            # per-b double buffers
            x_sb = [xpool.tile([C_IN + 1, T], F32, tag="x", name=f"xsb{i}") for i in range(2)]
            xp_sb = [xppool.tile([NC, CH * C_IN], F32, tag="xp", name=f"xpsb{i}") for i in range(2)]
            xwt_sb = [xppool.tile([C_IN + 1, NC], F32, tag="xwt", name=f"xwtsb{i}") for i in range(2)]
            for i in range(2):
                nc.vector.memset(x_sb[i][C_IN : C_IN + 1, :], 1.0)
                nc.vector.memset(xwt_sb[i][C_IN : C_IN + 1, :], 1.0)

            cs_b = []  # per-b (cos, sin, nsin) [128, 64]
            st_b = []  # per-b state tiles
            for b in range(BPC):
                cs_b.append(
                    (
                        pb.tile([NC, 64], F32, tag=f"cos{b}", name=f"cos{b}"),
                        pb.tile([NC, 64], F32, tag=f"sin{b}", name=f"sin{b}"),
                        pb.tile([NC, 64], F32, tag=f"nsin{b}", name=f"nsin{b}"),
                    )
                )
                st_b.append(stp.tile([128, STF], F32, tag=f"st{b}", name=f"st{b}"))

            lq_all = cp.tile([BPC, 2 * STF], F32, tag="lqall")
            mix = cp.tile([BPC, STF], F32, tag="mix")
            B_q = cp.tile([BPC, STF], F32, tag="Bq")
            qfeat = cp.tile([BPC, 19], F32, tag="qfeat")
            nc.vector.memset(qfeat[:, 18:19], 1.0)

            # ================= classical per-b =================
            for b in range(BPC):
                xb = x_sb[b % 2]
                nc.sync.dma_start(out=xb[0:C_IN, :], in_=xs[b, :, :])

                th = thpool.tile([128, T], F32, tag="th")
                for blk in range(4):
                    hp = ps_h.tile([128, 512], F32, tag="hp")
                    nc.tensor.matmul(
                        hp,
                        wfb_s,
                        xb[:, blk * 512 : (blk + 1) * 512],
                        start=True,
                        stop=True,
                    )
                    nc.scalar.activation(
                        th[:, blk * 512 : (blk + 1) * 512], hp, AF.Tanh
                    )
                    sc = ps_s.tile([1, 512], F32, tag="sc")
                    nc.tensor.matmul(
                        sc,
                        aw2_s,
                        th[:, blk * 512 : (blk + 1) * 512],
                        start=True,
                        stop=True,
                    )
                    # PSUM -> SBUF staging (DMA cannot read PSUM), then
                    # DMA-scatter [1, (32 nc, 16)] -> [32 partitions, 16]
                    ssc = sm.tile([1, 512], F32, tag="ssc", name="ssc")
                    if blk % 2 == 0:
                        nc.vector.tensor_copy(ssc, sc)
                    else:
                        nc.scalar.copy(ssc, sc)
                    g, bb = b // 8, b % 8
                    src = ssc.rearrange("p (n k) -> p n k", n=32, k=CH)
                    dst = sc_g[g][blk * 32 : (blk + 1) * 32, bb * CH : (bb + 1) * CH]
                    nc.sync.dma_start(out=dst, in_=src)

                # ---- group softmax + per-b chunk path, after each group of 8
                if b % 8 == 7:
                    g = b // 8
                    nc.scalar.activation(esc_g[g], sc_g[g], AF.Exp)
                    ssum = sm.tile([NC, 8], F32, tag="ssum")
                    nc.vector.tensor_reduce(
                        ssum,
                        esc_g[g].rearrange("p (n k) -> p n k", n=8, k=CH),
                        AX.X,
                        ALU.add,
                    )
                    rsum = sm.tile([NC, 8], F32, tag="rsum")
                    nc.vector.reciprocal(rsum, ssum)
                    for bb in range(8):
                        nc.vector.tensor_scalar_mul(
                            w_g[g][:, bb * CH : (bb + 1) * CH],
                            esc_g[g][:, bb * CH : (bb + 1) * CH],
                            rsum[:, bb : bb + 1],
                        )

                    for bb in range(8):
                        bfull = g * 8 + bb
                        xpb = xp_sb[bfull % 2]
                        nc.sync.dma_start(out=xpb, in_=xp[bfull, :, :])
                        # xw[nc, c] = sum_k w[nc, k] * xperm[nc, k*64: k*64+64]
                        xw = sm.tile([NC, C_IN], F32, tag="xw")
                        nc.vector.tensor_scalar_mul(
                            xw,
                            xpb[:, 0:C_IN],
                            w_g[g][:, bb * CH : bb * CH + 1],
                        )
                        for k in range(1, CH):
                            nc.vector.scalar_tensor_tensor(
                                xw,
                                xpb[:, k * C_IN : (k + 1) * C_IN],
                                w_g[g][:, bb * CH + k : bb * CH + k + 1],
                                xw,
                                ALU.mult,
                                ALU.add,
                            )
                        # transpose -> [64, 128]
                        xwt_ps = ps_m.tile([C_IN, NC], F32, tag="m")
                        nc.tensor.transpose(xwt_ps, xw, idn_s)
                        xwt = xwt_sb[bfull % 2]
                        nc.vector.tensor_copy(xwt[0:C_IN, :], xwt_ps)
                        # chunksT halves + params
                        cht = [None, None]
                        for h in range(2):
                            chp = ps_m.tile([128, NC], F32, tag="m")
                            nc.tensor.matmul(
                                chp,
                                ewb_s[:, h * 128 : (h + 1) * 128],
                                xwt,
                                start=True,
                                stop=True,
                            )
                            cht[h] = sm.tile([128, NC], F32, tag=f"cht{h}", name=f"cht{h}")
                            nc.vector.tensor_copy(cht[h], chp)
                        par = ps_t.tile([NC, 60], F32, tag="t")
                        nc.tensor.matmul(
                            par, cht[0], pjw_s[:, 0:60], start=True, stop=False
                        )
                        nc.tensor.matmul(
                            par, cht[1], pjw_s[:, 60:120], start=False, stop=False
                        )
                        nc.tensor.matmul(
                            par, ones, pjb_s, start=False, stop=True
                        )
                        co, si, ns = cs_b[bfull]
                        # theta = sigmoid(z); cos(theta/2) = sin(theta/2 + pi/2)
                        nc.scalar.activation(par, par, AF.Sigmoid)
                        nc.scalar.activation(
                            co[:, 0:60], par, AF.Sin,
                            bias=float(np.pi / 2), scale=0.5,
                        )
                        nc.scalar.activation(
                            si[:, 0:60], par, AF.Sin, bias=0.0, scale=0.5
                        )
                        nc.scalar.activation(
                            ns[:, 0:60], par, AF.Sin, bias=0.0, scale=-0.5
                        )

            # ================= quantum stage 1 =================
            for b in range(BPC):
                st = st_b[b]
                nc.vector.memset(st, 0.0)
                nc.vector.memset(st[:, 0:1], 1.0)
                co, si, ns = cs_b[b]

                def col(j, kind, co=co, si=si, ns=ns):
                    t = {"c": co, "s": si, "n": ns}[kind]
                    return t[:, j : j + 1]

                Bsc = B_dve if b % 2 == 0 else B_dve2
                emit_ansatz(nc.vector, st, Bsc, col, 2, sparse=True)

                # ---- LCU: 3 matmuls [K=128 nc] ----
                r0 = ps_t.tile([1, STF], F32, tag="t")
                nc.tensor.matmul(r0, cf2_s[:, 0:1], st, start=True, stop=True)
                m2a = ps_t.tile([1, DIM], F32, tag="t")
                nc.tensor.matmul(
                    m2a, cf2_s[:, 1:2], st[:, DIM:STF], start=True, stop=True
                )
                m2b = ps_t.tile([1, DIM], F32, tag="t")
                nc.tensor.matmul(
                    m2b, cf2_s[:, 1:2], st[:, 0:DIM], start=True, stop=True
                )
                # PSUM -> SBUF staging row [1, 256] = [r0 | m2a | m2b]
                lst = sm.tile([1, 2 * STF], F32, tag="lst", name="lst")
                nc.vector.tensor_copy(lst[:, 0:STF], r0)
                nc.vector.tensor_copy(lst[:, STF : STF + DIM], m2a)
                nc.vector.tensor_copy(lst[:, STF + DIM : 2 * STF], m2b)
                nc.sync.dma_start(out=lq_all[b : b + 1, :], in_=lst)

            # mixed = row0 -/+ m2 ; normalize
            nc.vector.tensor_tensor(
                mix[:, 0:DIM], lq_all[:, 0:DIM], lq_all[:, STF : STF + DIM],
                ALU.subtract,
            )
            nc.vector.tensor_tensor(
                mix[:, DIM:STF], lq_all[:, DIM:STF],
                lq_all[:, STF + DIM : 2 * STF], ALU.add,
            )
            sqs = sm.tile([BPC, STF], F32, tag="sqs")
            ss = sm.tile([BPC, 1], F32, tag="ss")
            nc.vector.tensor_tensor(sqs, mix, mix, ALU.mult)
            nc.vector.tensor_reduce(ss, sqs, AX.X, ALU.add)
            sd = sm.tile([BPC, 1], F32, tag="sd")
            nc.scalar.activation(sd, ss, AF.Sqrt)
            rn = sm.tile([BPC, 1], F32, tag="rn")
            nc.vector.reciprocal(rn, sd)
            nc.vector.tensor_scalar_mul(mix, mix, rn)

            # ================= qff ansatz (shared params) =================
            def qcol(j, kind):
                t = {"c": qfc_s, "s": qfs_s, "n": qfn_s}[kind]
                return t[:, j : j + 1]

            emit_ansatz(nc.vector, mix, B_q, qcol, 1, sparse=False)

            # ================= expvals -> qfeat [BPC, 18] =================
            scr = sm.tile([BPC, DIM], F32, tag="scr")
            scr2 = sm.tile([BPC, DIM], F32, tag="scr2")
            tmp1 = sm.tile([BPC, 1], F32, tag="tmp1")
            tmp2 = sm.tile([BPC, 1], F32, tag="tmp2")
            yr2 = sm.tile([BPC, 2], F32, tag="yr2")

            def clike(dst, off, ref):
                """contiguous view of dst at elem offset off, shaped like ref's
                free dims"""
                counts = [d[1] for d in ref.ap[1:]]
                dims = []
                stride = 1
                for c in reversed(counts):
                    dims.insert(0, [stride, c])
                    stride *= c
                return bass.AP(
                    tensor=dst.tensor, offset=dst.offset + off,
                    ap=[list(dst.ap[0])] + dims,
                )

            def prod(dst, off, a, b):
                nc.vector.tensor_tensor(clike(dst, off, a), a, b, ALU.mult)

            for i in range(NQ):
                p = 5 - i
                v = lambda ri, k: amp_view(mix, ri, {p: k})
                # X_i = 2 * sum(s0*s1) over re+im  (x2 applied at the end)
                prod(scr, 0, v(0, 0), v(0, 1))
                prod(scr, 32, v(1, 0), v(1, 1))
                nc.vector.tensor_reduce(qfeat[:, i : i + 1], scr, AX.X, ALU.add)
                # Y_i = 2 * sum(r0*i1 - i0*r1)
                prod(scr, 0, v(0, 0), v(1, 1))
                prod(scr, 32, v(1, 0), v(0, 1))
                nc.vector.tensor_reduce(
                    yr2,
                    scr.rearrange("p (h q) -> p h q", h=2, q=32),
                    AX.X,
                    ALU.add,
                )
                nc.vector.tensor_tensor(
                    qfeat[:, 6 + i : 7 + i], yr2[:, 0:1], yr2[:, 1:2],
                    ALU.subtract,
                )
                # Z_i = sum|bit0|^2 - sum|bit1|^2
                prod(scr, 0, v(0, 0), v(0, 0))
                prod(scr, 32, v(1, 0), v(1, 0))
                prod(scr2, 0, v(0, 1), v(0, 1))
                prod(scr2, 32, v(1, 1), v(1, 1))
                nc.vector.tensor_reduce(tmp1, scr, AX.X, ALU.add)
                nc.vector.tensor_reduce(tmp2, scr2, AX.X, ALU.add)
                nc.vector.tensor_tensor(
                    qfeat[:, 12 + i : 13 + i], tmp1, tmp2, ALU.subtract
                )
            # x2 for the X and Y blocks
            nc.vector.tensor_scalar_mul(qfeat[:, 0:12], qfeat[:, 0:12], 2.0)

            # ================= tail =================
            qfT_ps = ps_m.tile([19, BPC], F32, tag="m")
            nc.tensor.transpose(qfT_ps, qfeat, idn_s[0:BPC, 0:BPC])
            qfT = sm.tile([19, BPC], F32, tag="qfTs")
            nc.vector.tensor_copy(qfT, qfT_ps)
            o1 = ps_t.tile([BPC, D], F32, tag="t")
            nc.tensor.matmul(o1, qfT, owb_s, start=True, stop=True)

            stats = sm.tile([BPC, 6], F32, tag="stats")
            nc.vector.bn_stats(stats, o1)
            mv = sm.tile([BPC, 2], F32, tag="mv")
            nc.vector.bn_aggr(mv, stats)
            sdv = sm.tile([BPC, 1], F32, tag="sdv")
            nc.scalar.activation(sdv, mv[:, 1:2], AF.Sqrt, bias=1e-5)
            rstd = sm.tile([BPC, 1], F32, tag="rstd")
            nc.vector.reciprocal(rstd, sdv)
            ln1 = sm.tile([BPC, D], F32, tag="ln1")
            nc.vector.tensor_scalar(
                ln1, o1, mv[:, 0:1], rstd, ALU.subtract, ALU.mult
            )
            ln2 = sm.tile([BPC, D], F32, tag="ln2")
            nc.vector.tensor_tensor(ln2, ln1, lng_s, ALU.mult)
            nc.vector.tensor_tensor(ln2, ln2, lnb_s, ALU.add)

            # cls layer 1
            lnT = [None, None]
            for h in range(2):
                lnT_ps = ps_m.tile([128, BPC], F32, tag="m")
                nc.tensor.transpose(
                    lnT_ps, ln2[:, h * 128 : (h + 1) * 128], idn_s[0:BPC, 0:BPC]
                )
                lnT[h] = sm.tile([128, BPC], F32, tag=f"lnT{h}", name=f"lnT{h}")
                nc.vector.tensor_copy(lnT[h], lnT_ps)
            h2p = ps_t.tile([BPC, D], F32, tag="t")
            nc.tensor.matmul(h2p, lnT[0], cw1_s[:, 0:D], start=True, stop=False)
            nc.tensor.matmul(
                h2p, lnT[1], cw1_s[:, D : 2 * D], start=False, stop=False
            )
            nc.tensor.matmul(
                h2p, ones[:, 0:BPC], cb1_s, start=False, stop=True
            )
            h2 = sm.tile([BPC, D], F32, tag="h2")
            nc.scalar.activation(h2, h2p, AF.Relu)

            # cls layer 2
            h2T = [None, None]
            for h in range(2):
                h2T_ps = ps_m.tile([128, BPC], F32, tag="m")
                nc.tensor.transpose(
                    h2T_ps, h2[:, h * 128 : (h + 1) * 128], idn_s[0:BPC, 0:BPC]
                )
                h2T[h] = sm.tile([128, BPC], F32, tag=f"h2T{h}", name=f"h2T{h}")
                nc.vector.tensor_copy(h2T[h], h2T_ps)
            lg = ps_t.tile([BPC, 2], F32, tag="t")
            nc.tensor.matmul(lg, h2T[0], cw2_s[:, 0:2], start=True, stop=False)
            nc.tensor.matmul(lg, h2T[1], cw2_s[:, 2:4], start=False, stop=False)
            nc.tensor.matmul(lg, ones[:, 0:BPC], cb2_s, start=False, stop=True)
            lgs = sm.tile([BPC, 2], F32, tag="lgs")
            nc.vector.tensor_copy(lgs, lg)
            nc.sync.dma_start(out=out[:, :], in_=lgs)

    if split_waits:
        _split_multi_waits(nc)
    return nc


_NC_CACHE = {}


def _get_program():
    if "nc" not in _NC_CACHE:
        _NC_CACHE["nc"] = build_program()
    return _NC_CACHE["nc"]


def host_prep(inputs):
    """Host-side parameter folding -> per-core input maps."""
    f32 = np.float32
    x = np.asarray(inputs["x"], f32)
    emb_w = np.asarray(inputs["emb_w"], np.float64)
    emb_b = np.asarray(inputs["emb_b"], np.float64)
    att_w1 = np.asarray(inputs["att_w1"], np.float64)
    att_b1 = np.asarray(inputs["att_b1"], np.float64)

    wfold = (emb_w @ att_w1).astype(f32)  # [64, 128]
    bfold = (emb_b @ att_w1 + att_b1).astype(f32)  # [128]
    wfb = np.concatenate([wfold, bfold[None, :]], 0)  # [65, 128]

    ewb = np.concatenate(
        [emb_w.astype(f32), emb_b.astype(f32)[None, :]], 0
    )  # [65, 256]

    pw = np.asarray(inputs["proj_w"], f32)  # [256, 60]
    pjw = np.concatenate([pw[0:128, :], pw[128:256, :]], 1)  # [128, 120]

    cr = np.asarray(inputs["mix_re"], np.float64)
    ci = np.asarray(inputs["mix_im"], np.float64)
    den = np.sqrt(cr * cr + ci * ci).sum() + 1e-8
    cf2 = np.stack([cr / den, ci / den], 1).astype(f32)  # [128, 2]

    qp = np.asarray(inputs["qff_params"], np.float64) * 0.5
    qfc = np.broadcast_to(np.cos(qp).astype(f32), (BPC, 30)).copy()
    qfs = np.broadcast_to(np.sin(qp).astype(f32), (BPC, 30)).copy()
    qfn = (-qfs).copy()

    owb = np.concatenate(
        [np.asarray(inputs["out_w"], f32), np.asarray(inputs["out_b"], f32)[None, :]],
        0,
    )  # [19, 256]
    lng = np.broadcast_to(np.asarray(inputs["ln_g"], f32), (BPC, D)).copy()
    lnb = np.broadcast_to(np.asarray(inputs["ln_b"], f32), (BPC, D)).copy()
    w1 = np.asarray(inputs["cls_w1"], f32)
    cw1 = np.concatenate([w1[0:128, :], w1[128:256, :]], 1)  # [128, 512]
    cb1 = np.asarray(inputs["cls_b1"], f32)[None, :]
    w2 = np.asarray(inputs["cls_w2"], f32)
    cw2 = np.concatenate([w2[0:128, :], w2[128:256, :]], 1)  # [128, 4]
    cb2 = np.asarray(inputs["cls_b2"], f32)[None, :]
    idn = np.eye(128, dtype=f32)
    pjb = np.asarray(inputs["proj_b"], f32)[None, :]

    shared = dict(
        wfb=wfb, aw2=np.asarray(inputs["att_w2"], f32), ewb=ewb, pjw=pjw,
        pjb=pjb, cf2=cf2, qfc=qfc, qfs=qfs, qfn=qfn, owb=owb, lng=lng,
        lnb=lnb, cw1=cw1, cb1=cb1, cw2=cw2, cb2=cb2, idn=idn,
    )

    in_maps = []
    for c in range(N_CORES):
        xc = x[c * BPC : (c + 1) * BPC]  # [16, 64, 2048]
        # xperm[b, nc, k*64+c] = x[b, c, nc*16+k]
        xp_c = np.ascontiguousarray(
            xc.reshape(BPC, C_IN, NC, CH).transpose(0, 2, 3, 1).reshape(
                BPC, NC, CH * C_IN
            )
        )
        m = dict(shared)
        m["xs"] = np.ascontiguousarray(xc)
        m["xp"] = xp_c
        in_maps.append(m)
    return in_maps


def kernel(**inputs):
    nc = _get_program()
    in_maps = host_prep(inputs)
    res = run_bass_kernel_spmd(nc, in_maps, core_ids=list(range(N_CORES)))
    outs = [res.results[c]["out"] for c in range(N_CORES)]
    return np.concatenate(outs, 0).astype(np.float32)


if __name__ == "__main__":
    nc = build_program()
    print("program built ok")



# revision 2
# speedup vs baseline: 1.7137x; 1.7137x over previous
"""Trainium2 Bass kernel for nn_ClassicalQuantumAttention (batched rewrite).

Data-parallel over batch: 128 batch elems -> 16 per NeuronCore x 8 cores.

Quantum stage is BATCHED: per engine-group g with NB batch elems, one state
tile ST [128 nc, 128*NB] with free index f = q*NB + b  (q = ri*64 + a,
ri = re/im, a = 6-bit amplitude).  Each gate is ~6 big tensor_tensor ops
over all NB elems at once; per-(nc,b) cos/sin coefficients are read via
stride-0 broadcast views of [128, 60*NB] coefficient tiles.  Groups run on
different engines (DVE / Pool) as independent pipelines.

qff ansatz + expvals are folded on host into 19 symmetric 128x128 matrices
M_i = Ureal^T P_i Ureal (M_18 = I for the squared norm): qfeat_i[b] =
v_b^T M_i v_b via 19 PE matmuls + one elementwise mul + a ones-matmul
partition reduction.  LCU mixing is one K=128 matmul per 512-wide chunk.
"""

import numpy as np
import sys

for _p in ("/opt/trn_rl_repo",):
    if _p not in sys.path:
        sys.path.insert(0, _p)

import concourse.bass as bass
import concourse.tile as tile
from concourse import mybir
from concourse.bass_utils import run_bass_kernel_spmd

F32 = mybir.dt.float32
ALU = mybir.AluOpType
AF = mybir.ActivationFunctionType
AX = mybir.AxisListType

N_CORES = 8
B_TOT = 128
BPC = B_TOT // N_CORES  # 16
C_IN = 64
T = 2048
D = 256
CH = 16
NC = T // CH  # 128
NQ = 6
DIM = 64

# (engine_attr, b_start, NB)
GROUPS = [("vector", 0, 12), ("gpsimd", 12, 4)]


def ansatz_gates(n_layers):
    gates = []
    idx = 0
    for _ in range(n_layers):
        for i in range(NQ):
            gates.append(("rx", i, idx))
            gates.append(("ry", i, idx + 1))
            gates.append(("rz", i, idx + 2))
            idx += 3
        for i in range(NQ):
            gates.append(("crx", (i, (i + 1) % NQ), idx))
            idx += 1
        for i in range(NQ - 1, -1, -1):
            gates.append(("crx", (i, (i - 1) % NQ), idx))
            idx += 1
    return gates


# --------------------------------------------------------------- AP helpers
def bv(t, NB, ri, fixed, ah=6, split_b=False):
    """Batched state view of t [128, 128*NB], f = q*NB + b, q = ri*64 + a.

    ri: 0/1 or None (both halves; requires full contiguity).
    fixed: {amp_bit: 0/1}.  ah: active high bits (L1 sparsity support).
    split_b: represent a single full run as [[NB, n],[1, NB]] for coefficient
    shape-matching.
    """
    part = t.ap[0]
    off = t.offset
    lo_active = 6 - ah
    dims = []  # inner-first
    run = [1, NB]
    for p in range(6):
        w = NB * (1 << p)
        if p in fixed:
            off += fixed[p] * w
            if run is not None:
                dims.append(run)
                run = None
        elif p < lo_active:
            if run is not None:
                dims.append(run)
                run = None
        else:
            if run is not None and run[0] * run[1] == w:
                run[1] *= 2
            elif run is not None:
                dims.append(run)
                run = [w, 2]
            else:
                run = [w, 2]
    if ri is None:
        w = NB * 64
        assert run is not None and run[0] * run[1] == w and not dims, "ri-merge"
        run[1] *= 2
    else:
        off += ri * NB * 64
    if run is not None:
        dims.append(run)
    dims = dims[::-1]
    if split_b and len(dims) == 1 and dims[0][0] == 1:
        n = dims[0][1] // NB
        dims = [[NB, n], [1, NB]]
    assert 1 <= len(dims) <= 2, f"bv dims {dims}"
    return bass.AP(tensor=t.tensor, offset=off, ap=[list(part)] + dims)


def counts_of(ap):
    return [d[1] for d in ap.ap[1:]]


def coeffv(t, elem_off, counts):
    """Stride-0 broadcast view of coefficient tile t at elem_off matching
    counts ([outer, inner] -> [[0, outer], [1, inner]])."""
    if len(counts) == 2:
        dims = [[0, counts[0]], [1, counts[1]]]
    else:
        dims = [[1, counts[0]]]
    return bass.AP(tensor=t.tensor, offset=t.offset + elem_off,
                   ap=[list(t.ap[0])] + dims)


def rawv(t, elem_off, dims):
    return bass.AP(tensor=t.tensor, offset=t.offset + elem_off,
                   ap=[list(t.ap[0])] + dims)


# rotation add tables: (out_ri, out_k, in_ri, in_k, op)
ROT_ADDS = {
    "rx": [(0, 0, 1, 1, ALU.add), (0, 1, 1, 0, ALU.add),
           (1, 0, 0, 1, ALU.subtract), (1, 1, 0, 0, ALU.subtract)],
    "ry": [(0, 0, 0, 1, ALU.subtract), (0, 1, 0, 0, ALU.add),
           (1, 0, 1, 1, ALU.subtract), (1, 1, 1, 0, ALU.add)],
    "rz": [(0, 0, 1, 0, ALU.add), (1, 0, 0, 0, ALU.subtract),
           (0, 1, 1, 1, ALU.subtract), (1, 1, 0, 1, ALU.add)],
}


def _split_multi_waits(nc):
    """Walrus build allows at most ONE sync-wait per instruction; hoist
    extra waits onto same-engine NoOps."""
    ctr = [0]
    for f in nc.m.functions:
        for b in f.blocks:
            new = []
            for inst in b.instructions:
                si = inst.sync_info
                if si is not None and len(si.on_wait) > 1:
                    waits = list(si.on_wait)
                    for w in waits[:-1]:
                        ctr[0] += 1
                        nop = mybir.InstNoOp(
                            name=f"wsplit-{ctr[0]}",
                            ins=[], outs=[],
                            engine=inst.engine,
                            sync_info=mybir.SyncInfo(on_wait=[w], on_update=[]),
                        )
                        new.append(nop)
                    inst.sync_info = mybir.SyncInfo(
                        on_wait=[waits[-1]], on_update=list(si.on_update)
                    )
                new.append(inst)
            b.instructions = new


# ----------------------------------------------------------------- program
def build_program(split_waits=True):
    nc = bass.Bass()

    for vconst in (float(np.pi / 2), 1e-5):
        t = nc.alloc_sbuf_tensor(f"const-f32-{vconst}", [128, 1], F32)
        nc.gpsimd.memset(t.ap(), vconst)
        nc.const_aps.aps[(F32, vconst)] = t.ap()
    nc.all_engine_barrier()

    xs = nc.declare_dram_parameter("xs", [BPC, C_IN, T], F32, isOutput=False)
    xq = nc.declare_dram_parameter("xq", [BPC, NC, CH * C_IN], F32, isOutput=False)
    wfb = nc.declare_dram_parameter("wfb", [C_IN + 1, 128], F32, isOutput=False)
    aw2 = nc.declare_dram_parameter("aw2", [128, 1], F32, isOutput=False)
    ewb = nc.declare_dram_parameter("ewb", [C_IN + 1, D], F32, isOutput=False)
    pjw = nc.declare_dram_parameter("pjw", [128, 120], F32, isOutput=False)
    pjb = nc.declare_dram_parameter("pjb", [1, 60], F32, isOutput=False)
    cf2 = nc.declare_dram_parameter("cf2", [NC, 2], F32, isOutput=False)
    mt = nc.declare_dram_parameter("mt", [128, 19 * 128], F32, isOutput=False)
    owb = nc.declare_dram_parameter("owb", [19, D], F32, isOutput=False)
    lng = nc.declare_dram_parameter("lng", [BPC, D], F32, isOutput=False)
    lnb = nc.declare_dram_parameter("lnb", [BPC, D], F32, isOutput=False)
    cw1 = nc.declare_dram_parameter("cw1", [128, 2 * D], F32, isOutput=False)
    cb1 = nc.declare_dram_parameter("cb1", [1, D], F32, isOutput=False)
    cw2 = nc.declare_dram_parameter("cw2", [128, 4], F32, isOutput=False)
    cb2 = nc.declare_dram_parameter("cb2", [1, 2], F32, isOutput=False)
    idn = nc.declare_dram_parameter("idn", [128, 128], F32, isOutput=False)
    out = nc.declare_dram_parameter("out", [BPC, 2], F32, isOutput=True)

    with tile.TileContext(nc) as tc:
        with (
            tc.tile_pool(name="const", bufs=1) as cp,
            tc.tile_pool(name="xbuf", bufs=2) as xpool,
            tc.tile_pool(name="xqbuf", bufs=2) as xqpool,
            tc.tile_pool(name="tanh", bufs=2) as thpool,
            tc.tile_pool(name="small", bufs=4) as sm,
            tc.tile_pool(name="ps_h", bufs=2, space="PSUM") as ps_h,
            tc.tile_pool(name="ps_s", bufs=2, space="PSUM") as ps_s,
            tc.tile_pool(name="ps_m", bufs=2, space="PSUM") as ps_m,
            tc.tile_pool(name="ps_t", bufs=2, space="PSUM") as ps_t,
        ):
            def cload(name, dram, shape):
                t = cp.tile(shape, F32, tag=name, name=name)
                nc.sync.dma_start(out=t, in_=dram[:, :])
                return t

            wfb_s = cload("wfb", wfb, [C_IN + 1, 128])
            aw2_s = cload("aw2", aw2, [128, 1])
            ewb_s = cload("ewb", ewb, [C_IN + 1, D])
            pjw_s = cload("pjw", pjw, [128, 120])
            pjb_s = cload("pjb", pjb, [1, 60])
            cf2_s = cload("cf2", cf2, [NC, 2])
            mt_s = cload("mt", mt, [128, 19 * 128])
            owb_s = cload("owb", owb, [19, D])
            lng_s = cload("lng", lng, [BPC, D])
            lnb_s = cload("lnb", lnb, [BPC, D])
            cw1_s = cload("cw1", cw1, [128, 2 * D])
            cb1_s = cload("cb1", cb1, [1, D])
            cw2_s = cload("cw2", cw2, [128, 4])
            cb2_s = cload("cb2", cb2, [1, 2])
            idn_s = cload("idn", idn, [128, 128])

            ones = cp.tile([1, 128], F32, tag="ones")
            nc.vector.memset(ones, 1.0)
            ones_col = cp.tile([128, 1], F32, tag="ones_col")
            nc.vector.memset(ones_col, 1.0)

            sc_g = [cp.tile([NC, 8 * CH], F32, tag=f"scg{g}", name=f"scg{g}") for g in range(2)]
            esc_g = [cp.tile([NC, 8 * CH], F32, tag=f"escg{g}", name=f"escg{g}") for g in range(2)]
            w_g = [cp.tile([NC, 8 * CH], F32, tag=f"wg{g}", name=f"wg{g}") for g in range(2)]

            # per-engine-group quantum tiles
            grp = []
            for gi_, (engname, b0, NB) in enumerate(GROUPS):
                g = dict(
                    eng=getattr(nc, engname), b0=b0, NB=NB,
                    ST=cp.tile([128, 128 * NB], F32, tag=f"ST{gi_}", name=f"ST{gi_}"),
                    TMP=cp.tile([128, 128 * NB], F32, tag=f"TMP{gi_}", name=f"TMP{gi_}"),
                    co=cp.tile([128, 60 * NB], F32, tag=f"co{gi_}", name=f"co{gi_}"),
                    si=cp.tile([128, 60 * NB], F32, tag=f"si{gi_}", name=f"si{gi_}"),
                    sexp=[cp.tile([128, 32 * NB], F32, tag=f"sx{gi_}{k}", name=f"sx{gi_}{k}") for k in range(2)],
                    cexp=[cp.tile([128, 32 * NB], F32, tag=f"cx{gi_}{k}", name=f"cx{gi_}{k}") for k in range(2)],
                )
                grp.append(g)

            x_sb = [xpool.tile([C_IN + 1, T], F32, tag="x", name=f"xsb{i}") for i in range(2)]
            xq_sb = [xqpool.tile([NC, CH * C_IN], F32, tag="xq", name=f"xqsb{i}") for i in range(2)]
            for i in range(2):
                nc.vector.memset(x_sb[i][C_IN : C_IN + 1, :], 1.0)

            xwt_sb = [sm.tile([C_IN + 1, NC], F32, tag=f"xwt{i}", name=f"xwt{i}") for i in range(2)]
            for i in range(2):
                nc.vector.memset(xwt_sb[i][C_IN : C_IN + 1, :], 1.0)

            lqsA = cp.tile([2, 512 * ((GROUPS[0][2] * 128 + 511) // 512)], F32, tag="lqsA")
            lqsB = cp.tile([2, 512 * ((GROUPS[1][2] * 128 + 511) // 512)], F32, tag="lqsB")
            vTr = cp.tile([128, BPC], F32, tag="vTr")
            vTi = cp.tile([128, BPC], F32, tag="vTi")
            vT = cp.tile([128, BPC], F32, tag="vT")
            prod = cp.tile([128, 19 * BPC], F32, tag="prod")
            qrow = cp.tile([1, 19 * BPC], F32, tag="qrow")
            qfT = cp.tile([19, BPC], F32, tag="qfT")
            ssc = cp.tile([BPC, 1], F32, tag="ssc")
            rss = cp.tile([BPC, 1], F32, tag="rss")

            # ===================== classical =====================
            for b in range(BPC):
                xb = x_sb[b % 2]
                nc.sync.dma_start(out=xb[0:C_IN, :], in_=xs[b, :, :])

                for blk in range(4):
                    hp = ps_h.tile([128, 512], F32, tag="hp")
                    nc.tensor.matmul(hp, wfb_s, xb[:, blk * 512 : (blk + 1) * 512],
                                     start=True, stop=True)
                    th = thpool.tile([128, 512], F32, tag="th")
                    nc.scalar.activation(th, hp, AF.Tanh)
                    sc = ps_s.tile([1, 512], F32, tag="sc")
                    nc.tensor.matmul(sc, aw2_s, th, start=True, stop=True)
                    ssc_t = sm.tile([1, 512], F32, tag="sscb", name="sscb")
                    if blk % 2 == 0:
                        nc.vector.tensor_copy(ssc_t, sc)
                    else:
                        nc.scalar.copy(ssc_t, sc)
                    gg, bb = b // 8, b % 8
                    src = ssc_t.rearrange("p (n k) -> p n k", n=32, k=CH)
                    dst = sc_g[gg][blk * 32 : (blk + 1) * 32, bb * CH : (bb + 1) * CH]
                    nc.sync.dma_start(out=dst, in_=src)

                if b % 8 == 7:
                    gg = b // 8
                    nc.scalar.activation(esc_g[gg], sc_g[gg], AF.Exp)
                    ssum = sm.tile([NC, 8], F32, tag="ssum")
                    nc.vector.tensor_reduce(
                        ssum, esc_g[gg].rearrange("p (n k) -> p n k", n=8, k=CH),
                        AX.X, ALU.add)
                    rsum = sm.tile([NC, 8], F32, tag="rsum")
                    nc.vector.reciprocal(rsum, ssum)
                    for bb in range(8):
                        nc.vector.tensor_scalar_mul(
                            w_g[gg][:, bb * CH : (bb + 1) * CH],
                            esc_g[gg][:, bb * CH : (bb + 1) * CH],
                            rsum[:, bb : bb + 1])

                    for bb in range(8):
                        bfull = gg * 8 + bb
                        xqb = xq_sb[bfull % 2]
                        nc.sync.dma_start(out=xqb, in_=xq[bfull, :, :])
                        # xw[nc, c] = sum_k w[nc,k] * xq[nc, c*16+k]
                        tmpxw = sm.tile([NC, CH * C_IN], F32, tag="tmpxw")
                        wv = rawv(w_g[gg], bb * CH, [[0, C_IN], [1, CH]])
                        nc.vector.tensor_tensor(
                            rawv(tmpxw, 0, [[CH, C_IN], [1, CH]]),
                            rawv(xqb, 0, [[CH, C_IN], [1, CH]]),
                            wv, ALU.mult)
                        xw = sm.tile([NC, C_IN], F32, tag="xw")
                        nc.vector.tensor_reduce(
                            xw, tmpxw.rearrange("p (c k) -> p c k", c=C_IN, k=CH),
                            AX.X, ALU.add)
                        xwt_ps = ps_m.tile([C_IN, NC], F32, tag="m")
                        nc.tensor.transpose(xwt_ps, xw, idn_s)
                        xwt = xwt_sb[bfull % 2]
                        nc.scalar.copy(xwt[0:C_IN, :], xwt_ps)
                        cht = [None, None]
                        for h in range(2):
                            chp = ps_m.tile([128, NC], F32, tag="m")
                            nc.tensor.matmul(chp, ewb_s[:, h * 128 : (h + 1) * 128],
                                             xwt, start=True, stop=True)
                            cht[h] = sm.tile([128, NC], F32, tag=f"cht{h}", name=f"cht{h}")
                            nc.scalar.copy(cht[h], chp)
                        par = ps_t.tile([NC, 60], F32, tag="t")
                        nc.tensor.matmul(par, cht[0], pjw_s[:, 0:60], start=True, stop=False)
                        nc.tensor.matmul(par, cht[1], pjw_s[:, 60:120], start=False, stop=False)
                        nc.tensor.matmul(par, ones, pjb_s, start=False, stop=True)
                        par_s = sm.tile([NC, 60], F32, tag="pars", name="pars")
                        nc.scalar.activation(par_s, par, AF.Sigmoid)
                        # group-local coefficient write (strided, col j*NB+b')
                        for g in grp:
                            if g["b0"] <= bfull < g["b0"] + g["NB"]:
                                bl = bfull - g["b0"]
                                NB = g["NB"]
                                nc.scalar.activation(
                                    rawv(g["co"], bl, [[NB, 60]]), par_s,
                                    AF.Sin, bias=float(np.pi / 2), scale=0.5)
                                nc.scalar.activation(
                                    rawv(g["si"], bl, [[NB, 60]]), par_s,
                                    AF.Sin, bias=0.0, scale=0.5)
                                break

            # ===================== quantum stage 1 (batched) =====================
            for g in grp:
                eng, NB = g["eng"], g["NB"]
                eng.memset(g["ST"], 0.0)
                eng.memset(g["ST"][:, 0:NB], 1.0)

            gates = ansatz_gates(2)
            crx_ctr = 0
            for gi, (kind, loc, j) in enumerate(gates):
                ah = (gi // 3) + 1 if (kind != "crx" and gi < 18) else 6
                for g in grp:
                    eng, NB, ST, TMP = g["eng"], g["NB"], g["ST"], g["TMP"]
                    co, si = g["co"], g["si"]
                    if kind != "crx":
                        p = 5 - loc
                        if ah == 6:
                            sv = bv(ST, NB, None, {}, 6, split_b=True)
                            tv = bv(TMP, NB, None, {}, 6, split_b=True)
                            cts = counts_of(sv)
                            eng.tensor_tensor(tv, sv, coeffv(si, j * NB, cts), ALU.mult)
                            eng.tensor_tensor(sv, sv, coeffv(co, j * NB, cts), ALU.mult)
                        else:
                            for ri in (0, 1):
                                sv = bv(ST, NB, ri, {}, ah)
                                tv = bv(TMP, NB, ri, {}, ah)
                                cts = counts_of(sv)
                                eng.tensor_tensor(tv, sv, coeffv(si, j * NB, cts), ALU.mult)
                                eng.tensor_tensor(sv, sv, coeffv(co, j * NB, cts), ALU.mult)
                        for (oR, oK, iR, iK, op) in ROT_ADDS[kind]:
                            ov = bv(ST, NB, oR, {p: oK}, ah)
                            iv = bv(TMP, NB, iR, {p: iK}, ah)
                            eng.tensor_tensor(ov, ov, iv, op)
                    else:
                        wc, wt = loc
                        pc, pt = 5 - wc, 5 - wt
                        sx = g["sexp"][crx_ctr % 2]
                        cx = g["cexp"][crx_ctr % 2]
                        nc.scalar.copy(rawv(sx, 0, [[NB, 32], [1, NB]]),
                                       coeffv(si, j * NB, [32, NB]))
                        nc.scalar.copy(rawv(cx, 0, [[NB, 32], [1, NB]]),
                                       coeffv(co, j * NB, [32, NB]))
                        for ri in (0, 1):
                            sv = bv(ST, NB, ri, {pc: 1})
                            tv = bv(TMP, NB, ri, {pc: 1})
                            cts = counts_of(sv)
                            eng.tensor_tensor(tv, sv, coeffv(sx, 0, cts), ALU.mult)
                        for ri in (0, 1):
                            sv = bv(ST, NB, ri, {pc: 1})
                            cts = counts_of(sv)
                            eng.tensor_tensor(sv, sv, coeffv(cx, 0, cts), ALU.mult)
                        for kt in (0, 1):
                            ov = bv(ST, NB, 0, {pc: 1, pt: kt})
                            iv = bv(TMP, NB, 1, {pc: 1, pt: 1 - kt})
                            eng.tensor_tensor(ov, ov, iv, ALU.add)
                            ov = bv(ST, NB, 1, {pc: 1, pt: kt})
                            iv = bv(TMP, NB, 0, {pc: 1, pt: 1 - kt})
                            eng.tensor_tensor(ov, ov, iv, ALU.subtract)
                if kind == "crx":
                    crx_ctr += 1

            # ===================== LCU (one matmul per 512 chunk) ===============
            for g, lqs in ((grp[0], lqsA), (grp[1], lqsB)):
                NB = g["NB"]
                w = 128 * NB
                c0 = 0
                while c0 < w:
                    cw = min(512, w - c0)
                    lp = ps_s.tile([2, 512], F32, tag="sc")
                    nc.tensor.matmul(lp[:, 0:cw], cf2_s, g["ST"][:, c0 : c0 + cw],
                                     start=True, stop=True)
                    nc.scalar.copy(lqs[:, c0 : c0 + cw], lp[:, 0:cw])
                    c0 += cw

            # scatter rows into vTr / vTi (ri-swapped)
            for g, lqs in ((grp[0], lqsA), (grp[1], lqsB)):
                NB, b0 = g["NB"], g["b0"]
                nc.sync.dma_start(
                    out=vTr[:, b0 : b0 + NB],
                    in_=rawv(lqs[0:1, 0:1], 0, [[NB, 128], [1, NB]]))
                nc.sync.dma_start(
                    out=vTi[0:64, b0 : b0 + NB],
                    in_=rawv(lqs[1:2, 0:1], 64 * NB, [[NB, 64], [1, NB]]))
                nc.sync.dma_start(
                    out=vTi[64:128, b0 : b0 + NB],
                    in_=rawv(lqs[1:2, 0:1], 0, [[NB, 64], [1, NB]]))
            nc.vector.tensor_tensor(vT[0:64, :], vTr[0:64, :], vTi[0:64, :],
                                    ALU.subtract)
            nc.vector.tensor_tensor(vT[64:128, :], vTr[64:128, :], vTi[64:128, :],
                                    ALU.add)

            # ============== 19 quadratic forms  qfeat_i = v^T M_i v ==============
            t19 = ps_m.tile([128, 19 * BPC], F32, tag="m")
            for i in range(19):
                nc.tensor.matmul(t19[:, i * BPC : (i + 1) * BPC],
                                 mt_s[:, i * 128 : (i + 1) * 128], vT,
                                 start=True, stop=True)
            nc.vector.tensor_tensor(
                rawv(prod, 0, [[BPC, 19], [1, BPC]]),
                rawv(t19, 0, [[BPC, 19], [1, BPC]]),
                rawv(vT, 0, [[0, 19], [1, BPC]]), ALU.mult)
            qp_ps = ps_s.tile([1, 512], F32, tag="sc")
            nc.tensor.matmul(qp_ps[:, 0 : 19 * BPC], ones_col, prod,
                             start=True, stop=True)
            nc.scalar.copy(qrow, qp_ps[:, 0 : 19 * BPC])
            nc.sync.dma_start(out=qfT,
                              in_=qrow.rearrange("p (a b) -> p a b", a=19, b=BPC))
            nc.sync.dma_start(out=ssc,
                              in_=qfT[18:19, :].rearrange("p (a b) -> p a b", a=BPC, b=1))
            nc.vector.reciprocal(rss, ssc)

            # out head: o1 = (qfT^T @ owb) / ss   (row18 = ss pairs with out_b row)
            o1_ps = ps_t.tile([BPC, D], F32, tag="t")
            nc.tensor.matmul(o1_ps, qfT, owb_s, start=True, stop=True)
            o1 = sm.tile([BPC, D], F32, tag="o1")
            nc.vector.tensor_scalar_mul(o1, o1_ps, rss[:, 0:1])

            # LayerNorm
            stats = sm.tile([BPC, 6], F32, tag="stats")
            nc.vector.bn_stats(stats, o1)
            mv = sm.tile([BPC, 2], F32, tag="mv")
            nc.vector.bn_aggr(mv, stats)
            sdv = sm.tile([BPC, 1], F32, tag="sdv")
            nc.scalar.activation(sdv, mv[:, 1:2], AF.Sqrt, bias=1e-5)
            rstd = sm.tile([BPC, 1], F32, tag="rstd")
            nc.vector.reciprocal(rstd, sdv)
            ln1 = sm.tile([BPC, D], F32, tag="ln1")
            nc.vector.tensor_scalar(ln1, o1, mv[:, 0:1], rstd,
                                    ALU.subtract, ALU.mult)
            ln2 = sm.tile([BPC, D], F32, tag="ln2")
            nc.vector.tensor_tensor(ln2, ln1, lng_s, ALU.mult)
            nc.vector.tensor_tensor(ln2, ln2, lnb_s, ALU.add)

            # classifier
            lnT = [None, None]
            for h in range(2):
                lnT_ps = ps_m.tile([128, BPC], F32, tag="m")
                nc.tensor.transpose(lnT_ps, ln2[:, h * 128 : (h + 1) * 128],
                                    idn_s[0:BPC, 0:BPC])
                lnT[h] = sm.tile([128, BPC], F32, tag=f"lnT{h}", name=f"lnT{h}")
                nc.scalar.copy(lnT[h], lnT_ps)
            h2p = ps_t.tile([BPC, D], F32, tag="t")
            nc.tensor.matmul(h2p, lnT[0], cw1_s[:, 0:D], start=True, stop=False)
            nc.tensor.matmul(h2p, lnT[1], cw1_s[:, D : 2 * D], start=False, stop=False)
            nc.tensor.matmul(h2p, ones[:, 0:BPC], cb1_s, start=False, stop=True)
            h2 = sm.tile([BPC, D], F32, tag="h2")
            nc.scalar.activation(h2, h2p, AF.Relu)

            h2T = [None, None]
            for h in range(2):
                h2T_ps = ps_m.tile([128, BPC], F32, tag="m")
                nc.tensor.transpose(h2T_ps, h2[:, h * 128 : (h + 1) * 128],
                                    idn_s[0:BPC, 0:BPC])
                h2T[h] = sm.tile([128, BPC], F32, tag=f"h2T{h}", name=f"h2T{h}")
                nc.scalar.copy(h2T[h], h2T_ps)
            lg = ps_t.tile([BPC, 2], F32, tag="t")
            nc.tensor.matmul(lg, h2T[0], cw2_s[:, 0:2], start=True, stop=False)
            nc.tensor.matmul(lg, h2T[1], cw2_s[:, 2:4], start=False, stop=False)
            nc.tensor.matmul(lg, ones[:, 0:BPC], cb2_s, start=False, stop=True)
            lgs = sm.tile([BPC, 2], F32, tag="lgs")
            nc.vector.tensor_copy(lgs, lg)
            nc.sync.dma_start(out=out[:, :], in_=lgs)

    if split_waits:
        _split_multi_waits(nc)
    return nc


_NC_CACHE = {}


def _get_program():
    if "nc" not in _NC_CACHE:
        _NC_CACHE["nc"] = build_program()
    return _NC_CACHE["nc"]


# ----------------------------------------------------------------- host side
def _host_qff_matrices(qff_params, out_w):
    """19 symmetric 128x128 real matrices M_i = Ureal^T P_real_i Ureal."""
    qp = np.asarray(qff_params, np.float64)
    U = np.eye(DIM, dtype=np.complex128)

    def gate_1q(g2, wire):
        return np.kron(np.kron(np.eye(1 << wire), g2),
                       np.eye(1 << (NQ - 1 - wire)))

    def rx(t):
        c, s = np.cos(t / 2), np.sin(t / 2)
        return np.array([[c, -1j * s], [-1j * s, c]])

    def ry(t):
        c, s = np.cos(t / 2), np.sin(t / 2)
        return np.array([[c, -s], [s, c]])

    def rz(t):
        e = np.exp(-0.5j * t)
        return np.array([[e, 0], [0, np.conj(e)]])

    def crx_full(t, ctrl, tgt):
        G = np.eye(DIM, dtype=np.complex128)
        cb, tb = 5 - ctrl, 5 - tgt
        c, s = np.cos(t / 2), np.sin(t / 2)
        for a in range(DIM):
            if (a >> cb) & 1:
                G[a, a] = c
                G[a, a ^ (1 << tb)] = -1j * s
        return G

    for (kind, loc, j) in ansatz_gates(1):
        th = qp[j]
        if kind == "crx":
            G = crx_full(th, loc[0], loc[1])
        else:
            g2 = {"rx": rx, "ry": ry, "rz": rz}[kind](th)
            G = gate_1q(g2, loc)
        U = G @ U

    PX = np.array([[0, 1], [1, 0]], np.complex128)
    PY = np.array([[0, -1j], [1j, 0]], np.complex128)
    PZ = np.array([[1, 0], [0, -1]], np.complex128)

    mats = []
    for P in (PX, PY, PZ):
        for i in range(NQ):
            Pi = np.kron(np.kron(np.eye(1 << i), P), np.eye(1 << (NQ - 1 - i)))
            M = U.conj().T @ Pi @ U
            A, B = M.real, M.imag
            mats.append(np.block([[A, -B], [B, A]]))
    mats.append(np.eye(2 * DIM))
    MT = np.stack(mats, 0)  # [19, 128, 128]
    # lhsT[k, m] = M[m, k]; M symmetric -> store as-is
    return np.ascontiguousarray(
        MT.transpose(1, 0, 2).reshape(128, 19 * 128)).astype(np.float32)


def host_prep(inputs):
    f32 = np.float32
    x = np.asarray(inputs["x"], f32)
    emb_w = np.asarray(inputs["emb_w"], np.float64)
    emb_b = np.asarray(inputs["emb_b"], np.float64)
    att_w1 = np.asarray(inputs["att_w1"], np.float64)
    att_b1 = np.asarray(inputs["att_b1"], np.float64)

    wfold = (emb_w @ att_w1).astype(f32)
    bfold = (emb_b @ att_w1 + att_b1).astype(f32)
    wfb = np.concatenate([wfold, bfold[None, :]], 0)

    ewb = np.concatenate([emb_w.astype(f32), emb_b.astype(f32)[None, :]], 0)

    pw = np.asarray(inputs["proj_w"], f32)
    pjw = np.concatenate([pw[0:128, :], pw[128:256, :]], 1)

    cr = np.asarray(inputs["mix_re"], np.float64)
    ci = np.asarray(inputs["mix_im"], np.float64)
    den = np.sqrt(cr * cr + ci * ci).sum() + 1e-8
    cf2 = np.stack([cr / den, ci / den], 1).astype(f32)

    mt_m = _host_qff_matrices(inputs["qff_params"], inputs["out_w"])

    owb = np.concatenate(
        [np.asarray(inputs["out_w"], f32), np.asarray(inputs["out_b"], f32)[None, :]], 0)
    lng = np.broadcast_to(np.asarray(inputs["ln_g"], f32), (BPC, D)).copy()
    lnb = np.broadcast_to(np.asarray(inputs["ln_b"], f32), (BPC, D)).copy()
    w1 = np.asarray(inputs["cls_w1"], f32)
    cw1 = np.concatenate([w1[0:128, :], w1[128:256, :]], 1)
    cb1 = np.asarray(inputs["cls_b1"], f32)[None, :]
    w2 = np.asarray(inputs["cls_w2"], f32)
    cw2 = np.concatenate([w2[0:128, :], w2[128:256, :]], 1)
    cb2 = np.asarray(inputs["cls_b2"], f32)[None, :]
    idn = np.eye(128, dtype=f32)
    pjb = np.asarray(inputs["proj_b"], f32)[None, :]

    shared = dict(wfb=wfb, aw2=np.asarray(inputs["att_w2"], f32), ewb=ewb,
                  pjw=pjw, pjb=pjb, cf2=cf2, mt=mt_m, owb=owb, lng=lng,
                  lnb=lnb, cw1=cw1, cb1=cb1, cw2=cw2, cb2=cb2, idn=idn)

    in_maps = []
    for c in range(N_CORES):
        xc = x[c * BPC : (c + 1) * BPC]
        # xq[b, nc, cc*16+k] = x[b, cc, nc*16+k]  (c-major)
        xq_c = np.ascontiguousarray(
            xc.reshape(BPC, C_IN, NC, CH).transpose(0, 2, 1, 3).reshape(
                BPC, NC, C_IN * CH))
        m = dict(shared)
        m["xs"] = np.ascontiguousarray(xc)
        m["xq"] = xq_c
        in_maps.append(m)
    return in_maps


def kernel(**inputs):
    nc = _get_program()
    in_maps = host_prep(inputs)
    res = run_bass_kernel_spmd(nc, in_maps, core_ids=list(range(N_CORES)))
    outs = [res.results[c]["out"] for c in range(N_CORES)]
    return np.concatenate(outs, 0).astype(np.float32)


if __name__ == "__main__":
    nc = build_program()
    print("program built ok")


# revision 8
# speedup vs baseline: 2.0169x; 1.1769x over previous
"""Trainium2 Bass kernel for nn_ClassicalQuantumAttention (batched rewrite).

Data-parallel over batch: 128 batch elems -> 16 per NeuronCore x 8 cores.

Quantum stage is BATCHED: per engine-group g with NB batch elems, one state
tile ST [128 nc, 128*NB] with free index f = q*NB + b  (q = ri*64 + a,
ri = re/im, a = 6-bit amplitude).  Each gate is ~6 big tensor_tensor ops
over all NB elems at once; per-(nc,b) cos/sin coefficients are read via
stride-0 broadcast views of [128, 60*NB] coefficient tiles.  Groups run on
different engines (DVE / Pool) as independent pipelines.

qff ansatz + expvals are folded on host into 19 symmetric 128x128 matrices
M_i = Ureal^T P_i Ureal (M_18 = I for the squared norm): qfeat_i[b] =
v_b^T M_i v_b via 19 PE matmuls + one elementwise mul + a ones-matmul
partition reduction.  LCU mixing is one K=128 matmul per 512-wide chunk.
"""

import numpy as np
import sys

for _p in ("/opt/trn_rl_repo",):
    if _p not in sys.path:
        sys.path.insert(0, _p)

import concourse.bass as bass
import concourse.tile as tile
from concourse import mybir
from concourse.bass_utils import run_bass_kernel_spmd

F32 = mybir.dt.float32
F16 = mybir.dt.float16
ALU = mybir.AluOpType
AF = mybir.ActivationFunctionType
AX = mybir.AxisListType

N_CORES = 8
B_TOT = 128
BPC = B_TOT // N_CORES  # 16
C_IN = 64
T = 2048
D = 256
CH = 16
NC = T // CH  # 128
NQ = 6
DIM = 64

# (engine_attr, b_start, NB, state_dtype) — fp16 state enables the DVE 2x
# tensor_tensor mode; Pool stays fp32 (Q7 software path).
GROUPS = [("vector", 0, 13, F16), ("gpsimd", 13, 3, F32)]


def ansatz_gates(n_layers):
    gates = []
    idx = 0
    for _ in range(n_layers):
        for i in range(NQ):
            gates.append(("rx", i, idx))
            gates.append(("ry", i, idx + 1))
            gates.append(("rz", i, idx + 2))
            idx += 3
        for i in range(NQ):
            gates.append(("crx", (i, (i + 1) % NQ), idx))
            idx += 1
        for i in range(NQ - 1, -1, -1):
            gates.append(("crx", (i, (i - 1) % NQ), idx))
            idx += 1
    return gates


# --------------------------------------------------------------- AP helpers
def bv(t, NB, ri, fixed, ah=6, split_b=False):
    """Batched state view of t [128, 128*NB], f = q*NB + b, q = ri*64 + a.

    ri: 0/1 or None (both halves; requires full contiguity).
    fixed: {amp_bit: 0/1}.  ah: active high bits (L1 sparsity support).
    split_b: represent a single full run as [[NB, n],[1, NB]] for coefficient
    shape-matching.
    """
    part = t.ap[0]
    off = t.offset
    lo_active = 6 - ah
    dims = []  # inner-first
    run = [1, NB]
    for p in range(6):
        w = NB * (1 << p)
        if p in fixed:
            off += fixed[p] * w
            if run is not None:
                dims.append(run)
                run = None
        elif p < lo_active:
            if run is not None:
                dims.append(run)
                run = None
        else:
            if run is not None and run[0] * run[1] == w:
                run[1] *= 2
            elif run is not None:
                dims.append(run)
                run = [w, 2]
            else:
                run = [w, 2]
    if ri is None:
        w = NB * 64
        assert run is not None and run[0] * run[1] == w and not dims, "ri-merge"
        run[1] *= 2
    else:
        off += ri * NB * 64
    if run is not None:
        dims.append(run)
    dims = dims[::-1]
    if split_b and len(dims) == 1 and dims[0][0] == 1:
        n = dims[0][1] // NB
        dims = [[NB, n], [1, NB]]
    assert 1 <= len(dims) <= 2, f"bv dims {dims}"
    return bass.AP(tensor=t.tensor, offset=off, ap=[list(part)] + dims)


def counts_of(ap):
    return [d[1] for d in ap.ap[1:]]


def coeffv(t, elem_off, counts):
    """Stride-0 broadcast view of coefficient tile t at elem_off matching
    counts ([outer, inner] -> [[0, outer], [1, inner]])."""
    if len(counts) == 2:
        dims = [[0, counts[0]], [1, counts[1]]]
    else:
        dims = [[1, counts[0]]]
    return bass.AP(tensor=t.tensor, offset=t.offset + elem_off,
                   ap=[list(t.ap[0])] + dims)


def rawv(t, elem_off, dims):
    return bass.AP(tensor=t.tensor, offset=t.offset + elem_off,
                   ap=[list(t.ap[0])] + dims)


# rotation add tables: (out_ri, out_k, in_ri, in_k, op)
ROT_ADDS = {
    "rx": [(0, 0, 1, 1, ALU.add), (0, 1, 1, 0, ALU.add),
           (1, 0, 0, 1, ALU.subtract), (1, 1, 0, 0, ALU.subtract)],
    "ry": [(0, 0, 0, 1, ALU.subtract), (0, 1, 0, 0, ALU.add),
           (1, 0, 1, 1, ALU.subtract), (1, 1, 1, 0, ALU.add)],
    "rz": [(0, 0, 1, 0, ALU.add), (1, 0, 0, 0, ALU.subtract),
           (0, 1, 1, 1, ALU.subtract), (1, 1, 0, 1, ALU.add)],
}


def _split_multi_waits(nc):
    """Walrus build allows at most ONE sync-wait per instruction; hoist
    extra waits onto same-engine NoOps."""
    ctr = [0]
    for f in nc.m.functions:
        for b in f.blocks:
            new = []
            for inst in b.instructions:
                si = inst.sync_info
                if si is not None and len(si.on_wait) > 1:
                    waits = list(si.on_wait)
                    for w in waits[:-1]:
                        ctr[0] += 1
                        nop = mybir.InstNoOp(
                            name=f"wsplit-{ctr[0]}",
                            ins=[], outs=[],
                            engine=inst.engine,
                            sync_info=mybir.SyncInfo(on_wait=[w], on_update=[]),
                        )
                        new.append(nop)
                    inst.sync_info = mybir.SyncInfo(
                        on_wait=[waits[-1]], on_update=list(si.on_update)
                    )
                new.append(inst)
            b.instructions = new


# ----------------------------------------------------------------- program
def build_program(split_waits=True):
    nc = bass.Bass()

    for vconst in (float(np.pi / 2), 1e-5):
        t = nc.alloc_sbuf_tensor(f"const-f32-{vconst}", [128, 1], F32)
        nc.gpsimd.memset(t.ap(), vconst)
        nc.const_aps.aps[(F32, vconst)] = t.ap()
    nc.all_engine_barrier()

    xs = nc.declare_dram_parameter("xs", [BPC, C_IN, T], F32, isOutput=False)
    xq = nc.declare_dram_parameter("xq", [BPC, NC, CH * C_IN], F32, isOutput=False)
    wfb = nc.declare_dram_parameter("wfb", [C_IN + 1, 128], F32, isOutput=False)
    aw2 = nc.declare_dram_parameter("aw2", [128, 1], F32, isOutput=False)
    ewb = nc.declare_dram_parameter("ewb", [C_IN + 1, D], F32, isOutput=False)
    pjw = nc.declare_dram_parameter("pjw", [128, 120], F32, isOutput=False)
    pjb = nc.declare_dram_parameter("pjb", [1, 60], F32, isOutput=False)
    cf2 = nc.declare_dram_parameter("cf2", [NC, 2], F32, isOutput=False)
    mt = nc.declare_dram_parameter("mt", [128, 19 * 128], F32, isOutput=False)
    owb = nc.declare_dram_parameter("owb", [19, D], F32, isOutput=False)
    lng = nc.declare_dram_parameter("lng", [BPC, D], F32, isOutput=False)
    lnb = nc.declare_dram_parameter("lnb", [BPC, D], F32, isOutput=False)
    cw1 = nc.declare_dram_parameter("cw1", [128, 2 * D], F32, isOutput=False)
    cb1 = nc.declare_dram_parameter("cb1", [1, D], F32, isOutput=False)
    cw2 = nc.declare_dram_parameter("cw2", [128, 4], F32, isOutput=False)
    cb2 = nc.declare_dram_parameter("cb2", [1, 2], F32, isOutput=False)
    idn = nc.declare_dram_parameter("idn", [128, 128], F32, isOutput=False)
    out = nc.declare_dram_parameter("out", [BPC, 2], F32, isOutput=True)

    with tile.TileContext(nc) as tc:
        with (
            tc.tile_pool(name="const", bufs=1) as cp,
            tc.tile_pool(name="xbuf", bufs=2) as xpool,
            tc.tile_pool(name="xqbuf", bufs=2) as xqpool,
            tc.tile_pool(name="tanh", bufs=2) as thpool,
            tc.tile_pool(name="small", bufs=4) as sm,
            tc.tile_pool(name="ps_h", bufs=2, space="PSUM") as ps_h,
            tc.tile_pool(name="ps_s", bufs=2, space="PSUM") as ps_s,
            tc.tile_pool(name="ps_m", bufs=2, space="PSUM") as ps_m,
            tc.tile_pool(name="ps_t", bufs=2, space="PSUM") as ps_t,
        ):
            def cload(name, dram, shape):
                t = cp.tile(shape, F32, tag=name, name=name)
                nc.sync.dma_start(out=t, in_=dram[:, :])
                return t

            wfb_s = cload("wfb", wfb, [C_IN + 1, 128])
            aw2_s = cload("aw2", aw2, [128, 1])
            ewb_s = cload("ewb", ewb, [C_IN + 1, D])
            pjw_s = cload("pjw", pjw, [128, 120])
            pjb_s = cload("pjb", pjb, [1, 60])
            cf2_s = cload("cf2", cf2, [NC, 2])
            mt_s = cload("mt", mt, [128, 19 * 128])
            owb_s = cload("owb", owb, [19, D])
            lng_s = cload("lng", lng, [BPC, D])
            lnb_s = cload("lnb", lnb, [BPC, D])
            cw1_s = cload("cw1", cw1, [128, 2 * D])
            cb1_s = cload("cb1", cb1, [1, D])
            cw2_s = cload("cw2", cw2, [128, 4])
            cb2_s = cload("cb2", cb2, [1, 2])
            idn_s = cload("idn", idn, [128, 128])

            ones = cp.tile([1, 128], F32, tag="ones")
            nc.vector.memset(ones, 1.0)
            ones_col = cp.tile([128, 1], F32, tag="ones_col")
            nc.vector.memset(ones_col, 1.0)

            sc_g = [cp.tile([NC, 8 * CH], F32, tag=f"scg{g}", name=f"scg{g}") for g in range(2)]
            esc_g = [cp.tile([NC, 8 * CH], F32, tag=f"escg{g}", name=f"escg{g}") for g in range(2)]
            w_g = [cp.tile([NC, 8 * CH], F32, tag=f"wg{g}", name=f"wg{g}") for g in range(2)]

            # per-engine-group quantum tiles
            grp = []
            for gi_, (engname, b0, NB, sdt) in enumerate(GROUPS):
                g = dict(
                    eng=getattr(nc, engname), b0=b0, NB=NB, dt=sdt,
                    ST=cp.tile([128, 128 * NB], sdt, tag=f"ST{gi_}", name=f"ST{gi_}"),
                    TMP=cp.tile([128, 128 * NB], sdt, tag=f"TMP{gi_}", name=f"TMP{gi_}"),
                    co=cp.tile([128, 60 * NB], sdt, tag=f"co{gi_}", name=f"co{gi_}"),
                    si=cp.tile([128, 60 * NB], sdt, tag=f"si{gi_}", name=f"si{gi_}"),
                    sexp=[cp.tile([128, 32 * NB], sdt, tag=f"sx{gi_}{k}", name=f"sx{gi_}{k}") for k in range(2)],
                    cexp=[cp.tile([128, 32 * NB], sdt, tag=f"cx{gi_}{k}", name=f"cx{gi_}{k}") for k in range(2)],
                )
                grp.append(g)
            cf2_h = cp.tile([NC, 2], F16, tag="cf2h")
            nc.vector.tensor_copy(cf2_h, cf2_s)

            x_sb = [xpool.tile([C_IN + 1, T], F32, tag="x", name=f"xsb{i}") for i in range(2)]
            xq_sb = [xqpool.tile([NC, CH * C_IN], F32, tag="xq", name=f"xqsb{i}") for i in range(2)]
            for i in range(2):
                nc.vector.memset(x_sb[i][C_IN : C_IN + 1, :], 1.0)

            xwt_sb = [sm.tile([C_IN + 1, NC], F32, tag=f"xwt{i}", name=f"xwt{i}") for i in range(2)]
            for i in range(2):
                nc.vector.memset(xwt_sb[i][C_IN : C_IN + 1, :], 1.0)

            lqsA = cp.tile([2, 512 * ((GROUPS[0][2] * 128 + 511) // 512)], F32, tag="lqsA")
            lqsB = cp.tile([2, 512 * ((GROUPS[1][2] * 128 + 511) // 512)], F32, tag="lqsB")
            vTr = cp.tile([128, BPC], F32, tag="vTr")
            vTi = cp.tile([128, BPC], F32, tag="vTi")
            vT = cp.tile([128, BPC], F32, tag="vT")
            prod = cp.tile([128, 19 * BPC], F32, tag="prod")
            qrow = cp.tile([1, 19 * BPC], F32, tag="qrow")
            qfT = cp.tile([19, BPC], F32, tag="qfT")
            ssc = cp.tile([BPC, 1], F32, tag="ssc")
            rss = cp.tile([BPC, 1], F32, tag="rss")

            # ===================== classical =====================
            par_sb = [None] * 8
            for b in range(BPC):
                xb = x_sb[b % 2]
                nc.sync.dma_start(out=xb[0:C_IN, :], in_=xs[b, :, :])

                for blk in range(4):
                    hp = ps_h.tile([128, 512], F32, tag="hp")
                    nc.tensor.matmul(hp, wfb_s, xb[:, blk * 512 : (blk + 1) * 512],
                                     start=True, stop=True)
                    th = thpool.tile([128, 512], F32, tag="th")
                    nc.scalar.activation(th, hp, AF.Tanh)
                    sc = ps_s.tile([1, 512], F32, tag="sc")
                    nc.tensor.matmul(sc, aw2_s, th, start=True, stop=True)
                    ssc_t = sm.tile([1, 512], F32, tag="sscb", name="sscb")
                    if blk % 2 == 0:
                        nc.vector.tensor_copy(ssc_t, sc)
                    else:
                        nc.scalar.copy(ssc_t, sc)
                    gg, bb = b // 8, b % 8
                    src = ssc_t.rearrange("p (n k) -> p n k", n=32, k=CH)
                    dst = sc_g[gg][blk * 32 : (blk + 1) * 32, bb * CH : (bb + 1) * CH]
                    nc.sync.dma_start(out=dst, in_=src)

                if b % 8 == 7:
                    gg = b // 8
                    nc.scalar.activation(esc_g[gg], sc_g[gg], AF.Exp)
                    ssum = sm.tile([NC, 8], F32, tag="ssum")
                    nc.vector.tensor_reduce(
                        ssum, esc_g[gg].rearrange("p (n k) -> p n k", n=8, k=CH),
                        AX.X, ALU.add)
                    rsum = sm.tile([NC, 8], F32, tag="rsum")
                    nc.vector.reciprocal(rsum, ssum)
                    for bb in range(8):
                        nc.vector.tensor_scalar_mul(
                            w_g[gg][:, bb * CH : (bb + 1) * CH],
                            esc_g[gg][:, bb * CH : (bb + 1) * CH],
                            rsum[:, bb : bb + 1])

                    for bb in range(8):
                        bfull = gg * 8 + bb
                        xqb = xq_sb[bfull % 2]
                        nc.sync.dma_start(out=xqb, in_=xq[bfull, :, :])
                        # xw[nc, c] = sum_k w[nc,k] * xq[nc, c*16+k]
                        tmpxw = sm.tile([NC, CH * C_IN], F32, tag="tmpxw")
                        wv = rawv(w_g[gg], bb * CH, [[0, C_IN], [1, CH]])
                        nc.vector.tensor_tensor(
                            rawv(tmpxw, 0, [[CH, C_IN], [1, CH]]),
                            rawv(xqb, 0, [[CH, C_IN], [1, CH]]),
                            wv, ALU.mult)
                        xw = sm.tile([NC, C_IN], F32, tag="xw")
                        nc.vector.tensor_reduce(
                            xw, tmpxw.rearrange("p (c k) -> p c k", c=C_IN, k=CH),
                            AX.X, ALU.add)
                        xwt_ps = ps_m.tile([C_IN, NC], F32, tag="m")
                        nc.tensor.transpose(xwt_ps, xw, idn_s)
                        xwt = xwt_sb[bfull % 2]
                        nc.scalar.copy(xwt[0:C_IN, :], xwt_ps)
                        cht = [None, None]
                        for h in range(2):
                            chp = ps_m.tile([128, NC], F32, tag="m")
                            nc.tensor.matmul(chp, ewb_s[:, h * 128 : (h + 1) * 128],
                                             xwt, start=True, stop=True)
                            cht[h] = sm.tile([128, NC], F32, tag=f"cht{h}", name=f"cht{h}")
                            nc.scalar.copy(cht[h], chp)
                        par = ps_t.tile([NC, 60], F32, tag="t")
                        nc.tensor.matmul(par, cht[0], pjw_s[:, 0:60], start=True, stop=False)
                        nc.tensor.matmul(par, cht[1], pjw_s[:, 60:120], start=False, stop=False)
                        nc.tensor.matmul(par, ones, pjb_s, start=False, stop=True)
                        par_s = sm.tile([NC, 60], F32, tag=f"pars{bb}", name=f"pars{bb}")
                        nc.scalar.activation(par_s, par, AF.Sigmoid)
                        par_sb[bb] = par_s
                    # sins batched after all sigmoids (one ACT table swap)
                    for bb in range(8):
                        bfull = gg * 8 + bb
                        par_s = par_sb[bb]
                        for g in grp:
                            if g["b0"] <= bfull < g["b0"] + g["NB"]:
                                bl = bfull - g["b0"]
                                NB = g["NB"]
                                nc.scalar.activation(
                                    rawv(g["co"], bl, [[NB, 60]]), par_s,
                                    AF.Sin, bias=float(np.pi / 2), scale=0.5)
                                nc.scalar.activation(
                                    rawv(g["si"], bl, [[NB, 60]]), par_s,
                                    AF.Sin, bias=0.0, scale=0.5)
                                break

            # ===================== quantum stage 1 (batched) =====================
            for g in grp:
                eng, NB = g["eng"], g["NB"]
                eng.memset(g["ST"], 0.0)
                eng.memset(g["ST"][:, 0:NB], 1.0)

            gates = ansatz_gates(2)
            crx_ctr = 0
            for gi, (kind, loc, j) in enumerate(gates):
                ah = (gi // 3) + 1 if (kind != "crx" and gi < 18) else 6
                for g in grp:
                    eng, NB, ST, TMP = g["eng"], g["NB"], g["ST"], g["TMP"]
                    co, si = g["co"], g["si"]
                    if kind != "crx":
                        p = 5 - loc
                        if ah == 6:
                            sv = bv(ST, NB, None, {}, 6, split_b=True)
                            tv = bv(TMP, NB, None, {}, 6, split_b=True)
                            cts = counts_of(sv)
                            eng.tensor_tensor(tv, sv, coeffv(si, j * NB, cts), ALU.mult)
                            eng.tensor_tensor(sv, sv, coeffv(co, j * NB, cts), ALU.mult)
                        else:
                            for ri in (0, 1):
                                sv = bv(ST, NB, ri, {}, ah)
                                tv = bv(TMP, NB, ri, {}, ah)
                                cts = counts_of(sv)
                                eng.tensor_tensor(tv, sv, coeffv(si, j * NB, cts), ALU.mult)
                                eng.tensor_tensor(sv, sv, coeffv(co, j * NB, cts), ALU.mult)
                        for (oR, oK, iR, iK, op) in ROT_ADDS[kind]:
                            ov = bv(ST, NB, oR, {p: oK}, ah)
                            iv = bv(TMP, NB, iR, {p: iK}, ah)
                            eng.tensor_tensor(ov, ov, iv, op)
                    else:
                        wc, wt = loc
                        pc, pt = 5 - wc, 5 - wt
                        sx = g["sexp"][crx_ctr % 2]
                        cx = g["cexp"][crx_ctr % 2]
                        nc.scalar.copy(rawv(sx, 0, [[NB, 32], [1, NB]]),
                                       coeffv(si, j * NB, [32, NB]))
                        nc.scalar.copy(rawv(cx, 0, [[NB, 32], [1, NB]]),
                                       coeffv(co, j * NB, [32, NB]))
                        for ri in (0, 1):
                            sv = bv(ST, NB, ri, {pc: 1})
                            tv = bv(TMP, NB, ri, {pc: 1})
                            cts = counts_of(sv)
                            eng.tensor_tensor(tv, sv, coeffv(sx, 0, cts), ALU.mult)
                        for ri in (0, 1):
                            sv = bv(ST, NB, ri, {pc: 1})
                            cts = counts_of(sv)
                            eng.tensor_tensor(sv, sv, coeffv(cx, 0, cts), ALU.mult)
                        for kt in (0, 1):
                            ov = bv(ST, NB, 0, {pc: 1, pt: kt})
                            iv = bv(TMP, NB, 1, {pc: 1, pt: 1 - kt})
                            eng.tensor_tensor(ov, ov, iv, ALU.add)
                            ov = bv(ST, NB, 1, {pc: 1, pt: kt})
                            iv = bv(TMP, NB, 0, {pc: 1, pt: 1 - kt})
                            eng.tensor_tensor(ov, ov, iv, ALU.subtract)
                if kind == "crx":
                    crx_ctr += 1

            # ===================== LCU (one matmul per 512 chunk) ===============
            for g, lqs in ((grp[0], lqsA), (grp[1], lqsB)):
                NB = g["NB"]
                cfl = cf2_h if g["dt"] == F16 else cf2_s
                w = 128 * NB
                c0 = 0
                while c0 < w:
                    cw = min(512, w - c0)
                    lp = ps_s.tile([2, 512], F32, tag="sc")
                    nc.tensor.matmul(lp[:, 0:cw], cfl, g["ST"][:, c0 : c0 + cw],
                                     start=True, stop=True)
                    nc.scalar.copy(lqs[:, c0 : c0 + cw], lp[:, 0:cw])
                    c0 += cw

            # scatter rows into vTr / vTi (ri-swapped)
            for g, lqs in ((grp[0], lqsA), (grp[1], lqsB)):
                NB, b0 = g["NB"], g["b0"]
                nc.sync.dma_start(
                    out=vTr[:, b0 : b0 + NB],
                    in_=rawv(lqs[0:1, 0:1], 0, [[NB, 128], [1, NB]]))
                nc.sync.dma_start(
                    out=vTi[0:64, b0 : b0 + NB],
                    in_=rawv(lqs[1:2, 0:1], 64 * NB, [[NB, 64], [1, NB]]))
                nc.sync.dma_start(
                    out=vTi[64:128, b0 : b0 + NB],
                    in_=rawv(lqs[1:2, 0:1], 0, [[NB, 64], [1, NB]]))
            nc.vector.tensor_tensor(vT[0:64, :], vTr[0:64, :], vTi[0:64, :],
                                    ALU.subtract)
            nc.vector.tensor_tensor(vT[64:128, :], vTr[64:128, :], vTi[64:128, :],
                                    ALU.add)

            # ============== 19 quadratic forms  qfeat_i = v^T M_i v ==============
            t19 = ps_m.tile([128, 19 * BPC], F32, tag="m")
            for i in range(19):
                nc.tensor.matmul(t19[:, i * BPC : (i + 1) * BPC],
                                 mt_s[:, i * 128 : (i + 1) * 128], vT,
                                 start=True, stop=True)
            nc.vector.tensor_tensor(
                rawv(prod, 0, [[BPC, 19], [1, BPC]]),
                rawv(t19, 0, [[BPC, 19], [1, BPC]]),
                rawv(vT, 0, [[0, 19], [1, BPC]]), ALU.mult)
            qp_ps = ps_s.tile([1, 512], F32, tag="sc")
            nc.tensor.matmul(qp_ps[:, 0 : 19 * BPC], ones_col, prod,
                             start=True, stop=True)
            nc.scalar.copy(qrow, qp_ps[:, 0 : 19 * BPC])
            nc.sync.dma_start(out=qfT,
                              in_=qrow.rearrange("p (a b) -> p a b", a=19, b=BPC))
            nc.sync.dma_start(out=ssc,
                              in_=qfT[18:19, :].rearrange("p (a b) -> p a b", a=BPC, b=1))
            nc.vector.reciprocal(rss, ssc)

            # out head: o1 = (qfT^T @ owb) / ss   (row18 = ss pairs with out_b row)
            o1_ps = ps_t.tile([BPC, D], F32, tag="t")
            nc.tensor.matmul(o1_ps, qfT, owb_s, start=True, stop=True)
            o1 = sm.tile([BPC, D], F32, tag="o1")
            nc.vector.tensor_scalar_mul(o1, o1_ps, rss[:, 0:1])

            # LayerNorm
            stats = sm.tile([BPC, 6], F32, tag="stats")
            nc.vector.bn_stats(stats, o1)
            mv = sm.tile([BPC, 2], F32, tag="mv")
            nc.vector.bn_aggr(mv, stats)
            sdv = sm.tile([BPC, 1], F32, tag="sdv")
            nc.scalar.activation(sdv, mv[:, 1:2], AF.Sqrt, bias=1e-5)
            rstd = sm.tile([BPC, 1], F32, tag="rstd")
            nc.vector.reciprocal(rstd, sdv)
            ln1 = sm.tile([BPC, D], F32, tag="ln1")
            nc.vector.tensor_scalar(ln1, o1, mv[:, 0:1], rstd,
                                    ALU.subtract, ALU.mult)
            ln2 = sm.tile([BPC, D], F32, tag="ln2")
            nc.vector.tensor_tensor(ln2, ln1, lng_s, ALU.mult)
            nc.vector.tensor_tensor(ln2, ln2, lnb_s, ALU.add)

            # classifier
            lnT = [None, None]
            for h in range(2):
                lnT_ps = ps_m.tile([128, BPC], F32, tag="m")
                nc.tensor.transpose(lnT_ps, ln2[:, h * 128 : (h + 1) * 128],
                                    idn_s[0:BPC, 0:BPC])
                lnT[h] = sm.tile([128, BPC], F32, tag=f"lnT{h}", name=f"lnT{h}")
                nc.scalar.copy(lnT[h], lnT_ps)
            h2p = ps_t.tile([BPC, D], F32, tag="t")
            nc.tensor.matmul(h2p, lnT[0], cw1_s[:, 0:D], start=True, stop=False)
            nc.tensor.matmul(h2p, lnT[1], cw1_s[:, D : 2 * D], start=False, stop=False)
            nc.tensor.matmul(h2p, ones[:, 0:BPC], cb1_s, start=False, stop=True)
            h2 = sm.tile([BPC, D], F32, tag="h2")
            nc.scalar.activation(h2, h2p, AF.Relu)

            h2T = [None, None]
            for h in range(2):
                h2T_ps = ps_m.tile([128, BPC], F32, tag="m")
                nc.tensor.transpose(h2T_ps, h2[:, h * 128 : (h + 1) * 128],
                                    idn_s[0:BPC, 0:BPC])
                h2T[h] = sm.tile([128, BPC], F32, tag=f"h2T{h}", name=f"h2T{h}")
                nc.scalar.copy(h2T[h], h2T_ps)
            lg = ps_t.tile([BPC, 2], F32, tag="t")
            nc.tensor.matmul(lg, h2T[0], cw2_s[:, 0:2], start=True, stop=False)
            nc.tensor.matmul(lg, h2T[1], cw2_s[:, 2:4], start=False, stop=False)
            nc.tensor.matmul(lg, ones[:, 0:BPC], cb2_s, start=False, stop=True)
            lgs = sm.tile([BPC, 2], F32, tag="lgs")
            nc.vector.tensor_copy(lgs, lg)
            nc.sync.dma_start(out=out[:, :], in_=lgs)

    if split_waits:
        _split_multi_waits(nc)
    return nc


_NC_CACHE = {}


def _get_program():
    if "nc" not in _NC_CACHE:
        _NC_CACHE["nc"] = build_program()
    return _NC_CACHE["nc"]


# ----------------------------------------------------------------- host side
def _host_qff_matrices(qff_params, out_w):
    """19 symmetric 128x128 real matrices M_i = Ureal^T P_real_i Ureal."""
    qp = np.asarray(qff_params, np.float64)
    U = np.eye(DIM, dtype=np.complex128)

    def gate_1q(g2, wire):
        return np.kron(np.kron(np.eye(1 << wire), g2),
                       np.eye(1 << (NQ - 1 - wire)))

    def rx(t):
        c, s = np.cos(t / 2), np.sin(t / 2)
        return np.array([[c, -1j * s], [-1j * s, c]])

    def ry(t):
        c, s = np.cos(t / 2), np.sin(t / 2)
        return np.array([[c, -s], [s, c]])

    def rz(t):
        e = np.exp(-0.5j * t)
        return np.array([[e, 0], [0, np.conj(e)]])

    def crx_full(t, ctrl, tgt):
        G = np.eye(DIM, dtype=np.complex128)
        cb, tb = 5 - ctrl, 5 - tgt
        c, s = np.cos(t / 2), np.sin(t / 2)
        for a in range(DIM):
            if (a >> cb) & 1:
                G[a, a] = c
                G[a, a ^ (1 << tb)] = -1j * s
        return G

    for (kind, loc, j) in ansatz_gates(1):
        th = qp[j]
        if kind == "crx":
            G = crx_full(th, loc[0], loc[1])
        else:
            g2 = {"rx": rx, "ry": ry, "rz": rz}[kind](th)
            G = gate_1q(g2, loc)
        U = G @ U

    PX = np.array([[0, 1], [1, 0]], np.complex128)
    PY = np.array([[0, -1j], [1j, 0]], np.complex128)
    PZ = np.array([[1, 0], [0, -1]], np.complex128)

    mats = []
    for P in (PX, PY, PZ):
        for i in range(NQ):
            Pi = np.kron(np.kron(np.eye(1 << i), P), np.eye(1 << (NQ - 1 - i)))
            M = U.conj().T @ Pi @ U
            A, B = M.real, M.imag
            mats.append(np.block([[A, -B], [B, A]]))
    mats.append(np.eye(2 * DIM))
    MT = np.stack(mats, 0)  # [19, 128, 128]
    # lhsT[k, m] = M[m, k]; M symmetric -> store as-is
    return np.ascontiguousarray(
        MT.transpose(1, 0, 2).reshape(128, 19 * 128)).astype(np.float32)


def host_prep(inputs):
    f32 = np.float32
    x = np.asarray(inputs["x"], f32)
    emb_w = np.asarray(inputs["emb_w"], np.float64)
    emb_b = np.asarray(inputs["emb_b"], np.float64)
    att_w1 = np.asarray(inputs["att_w1"], np.float64)
    att_b1 = np.asarray(inputs["att_b1"], np.float64)

    wfold = (emb_w @ att_w1).astype(f32)
    bfold = (emb_b @ att_w1 + att_b1).astype(f32)
    wfb = np.concatenate([wfold, bfold[None, :]], 0)

    ewb = np.concatenate([emb_w.astype(f32), emb_b.astype(f32)[None, :]], 0)

    pw = np.asarray(inputs["proj_w"], f32)
    pjw = np.concatenate([pw[0:128, :], pw[128:256, :]], 1)

    cr = np.asarray(inputs["mix_re"], np.float64)
    ci = np.asarray(inputs["mix_im"], np.float64)
    den = np.sqrt(cr * cr + ci * ci).sum() + 1e-8
    cf2 = np.stack([cr / den, ci / den], 1).astype(f32)

    mt_m = _host_qff_matrices(inputs["qff_params"], inputs["out_w"])

    owb = np.concatenate(
        [np.asarray(inputs["out_w"], f32), np.asarray(inputs["out_b"], f32)[None, :]], 0)
    lng = np.broadcast_to(np.asarray(inputs["ln_g"], f32), (BPC, D)).copy()
    lnb = np.broadcast_to(np.asarray(inputs["ln_b"], f32), (BPC, D)).copy()
    w1 = np.asarray(inputs["cls_w1"], f32)
    cw1 = np.concatenate([w1[0:128, :], w1[128:256, :]], 1)
    cb1 = np.asarray(inputs["cls_b1"], f32)[None, :]
    w2 = np.asarray(inputs["cls_w2"], f32)
    cw2 = np.concatenate([w2[0:128, :], w2[128:256, :]], 1)
    cb2 = np.asarray(inputs["cls_b2"], f32)[None, :]
    idn = np.eye(128, dtype=f32)
    pjb = np.asarray(inputs["proj_b"], f32)[None, :]

    shared = dict(wfb=wfb, aw2=np.asarray(inputs["att_w2"], f32), ewb=ewb,
                  pjw=pjw, pjb=pjb, cf2=cf2, mt=mt_m, owb=owb, lng=lng,
                  lnb=lnb, cw1=cw1, cb1=cb1, cw2=cw2, cb2=cb2, idn=idn)

    in_maps = []
    for c in range(N_CORES):
        xc = x[c * BPC : (c + 1) * BPC]
        # xq[b, nc, cc*16+k] = x[b, cc, nc*16+k]  (c-major)
        xq_c = np.ascontiguousarray(
            xc.reshape(BPC, C_IN, NC, CH).transpose(0, 2, 1, 3).reshape(
                BPC, NC, C_IN * CH))
        m = dict(shared)
        m["xs"] = np.ascontiguousarray(xc)
        m["xq"] = xq_c
        in_maps.append(m)
    return in_maps


def kernel(**inputs):
    nc = _get_program()
    in_maps = host_prep(inputs)
    res = run_bass_kernel_spmd(nc, in_maps, core_ids=list(range(N_CORES)))
    outs = [res.results[c]["out"] for c in range(N_CORES)]
    return np.concatenate(outs, 0).astype(np.float32)


if __name__ == "__main__":
    nc = build_program()
    print("program built ok")


# revision 13
# speedup vs baseline: 2.2077x; 1.0946x over previous
"""Trainium2 Bass kernel for nn_ClassicalQuantumAttention (batched rewrite).

Data-parallel over batch: 128 batch elems -> 16 per NeuronCore x 8 cores.

Quantum stage is BATCHED: per engine-group g with NB batch elems, one state
tile ST [128 nc, 128*NB] with free index f = q*NB + b  (q = ri*64 + a,
ri = re/im, a = 6-bit amplitude).  Each gate is ~6 big tensor_tensor ops
over all NB elems at once; per-(nc,b) cos/sin coefficients are read via
stride-0 broadcast views of [128, 60*NB] coefficient tiles.  Groups run on
different engines (DVE / Pool) as independent pipelines.

qff ansatz + expvals are folded on host into 19 symmetric 128x128 matrices
M_i = Ureal^T P_i Ureal (M_18 = I for the squared norm): qfeat_i[b] =
v_b^T M_i v_b via 19 PE matmuls + one elementwise mul + a ones-matmul
partition reduction.  LCU mixing is one K=128 matmul per 512-wide chunk.
"""

import numpy as np
import sys

for _p in ("/opt/trn_rl_repo",):
    if _p not in sys.path:
        sys.path.insert(0, _p)

import concourse.bass as bass
import concourse.tile as tile
from concourse import mybir
from concourse.bass_utils import run_bass_kernel_spmd

F32 = mybir.dt.float32
F16 = mybir.dt.float16
ALU = mybir.AluOpType
AF = mybir.ActivationFunctionType
AX = mybir.AxisListType

N_CORES = 8
B_TOT = 128
BPC = B_TOT // N_CORES  # 16
C_IN = 64
T = 2048
D = 256
CH = 16
NC = T // CH  # 128
NQ = 6
DIM = 64

# (engine_attr, b_start, NB, state_dtype) — fp16 state enables the DVE 2x
# tensor_tensor mode; Pool stays fp32 (Q7 software path).
GROUPS = [("vector", 0, 13, F16), ("gpsimd", 13, 3, F32)]


def ansatz_gates(n_layers):
    gates = []
    idx = 0
    for _ in range(n_layers):
        for i in range(NQ):
            gates.append(("rx", i, idx))
            gates.append(("ry", i, idx + 1))
            gates.append(("rz", i, idx + 2))
            idx += 3
        for i in range(NQ):
            gates.append(("crx", (i, (i + 1) % NQ), idx))
            idx += 1
        for i in range(NQ - 1, -1, -1):
            gates.append(("crx", (i, (i - 1) % NQ), idx))
            idx += 1
    return gates


# --------------------------------------------------------------- AP helpers
def bv(t, NB, ri, fixed, ah=6, split_b=False):
    """Batched state view of t [128, 128*NB], f = q*NB + b, q = ri*64 + a.

    ri: 0/1 or None (both halves; requires full contiguity).
    fixed: {amp_bit: 0/1}.  ah: active high bits (L1 sparsity support).
    split_b: represent a single full run as [[NB, n],[1, NB]] for coefficient
    shape-matching.
    """
    part = t.ap[0]
    off = t.offset
    lo_active = 6 - ah
    dims = []  # inner-first
    run = [1, NB]
    for p in range(6):
        w = NB * (1 << p)
        if p in fixed:
            off += fixed[p] * w
            if run is not None:
                dims.append(run)
                run = None
        elif p < lo_active:
            if run is not None:
                dims.append(run)
                run = None
        else:
            if run is not None and run[0] * run[1] == w:
                run[1] *= 2
            elif run is not None:
                dims.append(run)
                run = [w, 2]
            else:
                run = [w, 2]
    if ri is None:
        w = NB * 64
        assert run is not None and run[0] * run[1] == w and not dims, "ri-merge"
        run[1] *= 2
    else:
        off += ri * NB * 64
    if run is not None:
        dims.append(run)
    dims = dims[::-1]
    if split_b and len(dims) == 1 and dims[0][0] == 1:
        n = dims[0][1] // NB
        dims = [[NB, n], [1, NB]]
    assert 1 <= len(dims) <= 2, f"bv dims {dims}"
    return bass.AP(tensor=t.tensor, offset=off, ap=[list(part)] + dims)


def counts_of(ap):
    return [d[1] for d in ap.ap[1:]]


def coeffv(t, elem_off, counts):
    """Stride-0 broadcast view of coefficient tile t at elem_off matching
    counts ([outer, inner] -> [[0, outer], [1, inner]])."""
    if len(counts) == 2:
        dims = [[0, counts[0]], [1, counts[1]]]
    else:
        dims = [[1, counts[0]]]
    return bass.AP(tensor=t.tensor, offset=t.offset + elem_off,
                   ap=[list(t.ap[0])] + dims)


def rawv(t, elem_off, dims):
    return bass.AP(tensor=t.tensor, offset=t.offset + elem_off,
                   ap=[list(t.ap[0])] + dims)


# rotation add tables: (out_ri, out_k, in_ri, in_k, op)
ROT_ADDS = {
    "rx": [(0, 0, 1, 1, ALU.add), (0, 1, 1, 0, ALU.add),
           (1, 0, 0, 1, ALU.subtract), (1, 1, 0, 0, ALU.subtract)],
    "ry": [(0, 0, 0, 1, ALU.subtract), (0, 1, 0, 0, ALU.add),
           (1, 0, 1, 1, ALU.subtract), (1, 1, 1, 0, ALU.add)],
    "rz": [(0, 0, 1, 0, ALU.add), (1, 0, 0, 0, ALU.subtract),
           (0, 1, 1, 1, ALU.subtract), (1, 1, 0, 1, ALU.add)],
}


def _split_multi_waits(nc):
    """Walrus build allows at most ONE sync-wait per instruction; hoist
    extra waits onto same-engine NoOps."""
    ctr = [0]
    for f in nc.m.functions:
        for b in f.blocks:
            new = []
            for inst in b.instructions:
                si = inst.sync_info
                if si is not None and len(si.on_wait) > 1:
                    waits = list(si.on_wait)
                    for w in waits[:-1]:
                        ctr[0] += 1
                        nop = mybir.InstNoOp(
                            name=f"wsplit-{ctr[0]}",
                            ins=[], outs=[],
                            engine=inst.engine,
                            sync_info=mybir.SyncInfo(on_wait=[w], on_update=[]),
                        )
                        new.append(nop)
                    inst.sync_info = mybir.SyncInfo(
                        on_wait=[waits[-1]], on_update=list(si.on_update)
                    )
                new.append(inst)
            b.instructions = new


# ----------------------------------------------------------------- program
def build_program(split_waits=True):
    nc = bass.Bass()

    for vconst in (float(np.pi / 2), 1e-5):
        t = nc.alloc_sbuf_tensor(f"const-f32-{vconst}", [128, 1], F32)
        nc.gpsimd.memset(t.ap(), vconst)
        nc.const_aps.aps[(F32, vconst)] = t.ap()
    nc.all_engine_barrier()

    xs = nc.declare_dram_parameter("xs", [BPC, C_IN, T], F16, isOutput=False)
    xq = nc.declare_dram_parameter("xq", [BPC, NC, CH * C_IN], F32, isOutput=False)
    wfb = nc.declare_dram_parameter("wfb", [C_IN + 1, 128], F16, isOutput=False)
    aw2 = nc.declare_dram_parameter("aw2", [128, 1], F16, isOutput=False)
    ewb = nc.declare_dram_parameter("ewb", [C_IN + 1, D], F16, isOutput=False)
    pjw = nc.declare_dram_parameter("pjw", [128, 120], F16, isOutput=False)
    pjb = nc.declare_dram_parameter("pjb", [1, 60], F16, isOutput=False)
    cf2 = nc.declare_dram_parameter("cf2", [NC, 2], F32, isOutput=False)
    mt = nc.declare_dram_parameter("mt", [128, 19 * 128], F32, isOutput=False)
    owb = nc.declare_dram_parameter("owb", [19, D], F32, isOutput=False)
    lng = nc.declare_dram_parameter("lng", [BPC, D], F32, isOutput=False)
    lnb = nc.declare_dram_parameter("lnb", [BPC, D], F32, isOutput=False)
    cw1 = nc.declare_dram_parameter("cw1", [128, 2 * D], F32, isOutput=False)
    cb1 = nc.declare_dram_parameter("cb1", [1, D], F32, isOutput=False)
    cw2 = nc.declare_dram_parameter("cw2", [128, 4], F32, isOutput=False)
    cb2 = nc.declare_dram_parameter("cb2", [1, 2], F32, isOutput=False)
    idn = nc.declare_dram_parameter("idn", [128, 128], F32, isOutput=False)
    out = nc.declare_dram_parameter("out", [BPC, 2], F32, isOutput=True)

    with tile.TileContext(nc) as tc:
        with (
            tc.tile_pool(name="const", bufs=1) as cp,
            tc.tile_pool(name="xbuf", bufs=2) as xpool,
            tc.tile_pool(name="xqbuf", bufs=2) as xqpool,
            tc.tile_pool(name="tanh", bufs=2) as thpool,
            tc.tile_pool(name="small", bufs=4) as sm,
            tc.tile_pool(name="ps_h", bufs=2, space="PSUM") as ps_h,
            tc.tile_pool(name="ps_s", bufs=2, space="PSUM") as ps_s,
            tc.tile_pool(name="ps_m", bufs=2, space="PSUM") as ps_m,
            tc.tile_pool(name="ps_t", bufs=2, space="PSUM") as ps_t,
        ):
            def cload(name, dram, shape, dt=F32):
                t = cp.tile(shape, dt, tag=name, name=name)
                nc.sync.dma_start(out=t, in_=dram[:, :])
                return t

            wfb_s = cload("wfb", wfb, [C_IN + 1, 128], F16)
            aw2_s = cload("aw2", aw2, [128, 1], F16)
            ewb_s = cload("ewb", ewb, [C_IN + 1, D], F16)
            pjw_s = cload("pjw", pjw, [128, 120], F16)
            pjb_s = cload("pjb", pjb, [1, 60], F16)
            cf2_s = cload("cf2", cf2, [NC, 2])
            mt_s = cload("mt", mt, [128, 19 * 128])
            owb_s = cload("owb", owb, [19, D])
            lng_s = cload("lng", lng, [BPC, D])
            lnb_s = cload("lnb", lnb, [BPC, D])
            cw1_s = cload("cw1", cw1, [128, 2 * D])
            cb1_s = cload("cb1", cb1, [1, D])
            cw2_s = cload("cw2", cw2, [128, 4])
            cb2_s = cload("cb2", cb2, [1, 2])
            idn_s = cload("idn", idn, [128, 128])

            ones = cp.tile([1, 128], F32, tag="ones")
            nc.vector.memset(ones, 1.0)
            ones_col = cp.tile([128, 1], F32, tag="ones_col")
            nc.vector.memset(ones_col, 1.0)
            ones_h = cp.tile([1, 128], F16, tag="ones_h")
            nc.vector.memset(ones_h, 1.0)

            sc_g = [cp.tile([NC, 8 * CH], F32, tag=f"scg{g}", name=f"scg{g}") for g in range(2)]
            esc_g = [cp.tile([NC, 8 * CH], F32, tag=f"escg{g}", name=f"escg{g}") for g in range(2)]
            w_g = [cp.tile([NC, 8 * CH], F32, tag=f"wg{g}", name=f"wg{g}") for g in range(2)]

            # per-engine-group quantum tiles
            grp = []
            for gi_, (engname, b0, NB, sdt) in enumerate(GROUPS):
                g = dict(
                    eng=getattr(nc, engname), b0=b0, NB=NB, dt=sdt,
                    ST=cp.tile([128, 128 * NB], sdt, tag=f"ST{gi_}", name=f"ST{gi_}"),
                    TMP=cp.tile([128, 128 * NB], sdt, tag=f"TMP{gi_}", name=f"TMP{gi_}"),
                    co=cp.tile([128, 60 * NB], sdt, tag=f"co{gi_}", name=f"co{gi_}"),
                    si=cp.tile([128, 60 * NB], sdt, tag=f"si{gi_}", name=f"si{gi_}"),
                    sexp=[cp.tile([128, 32 * NB], sdt, tag=f"sx{gi_}{k}", name=f"sx{gi_}{k}") for k in range(2)],
                    cexp=[cp.tile([128, 32 * NB], sdt, tag=f"cx{gi_}{k}", name=f"cx{gi_}{k}") for k in range(2)],
                )
                grp.append(g)
            cf2_h = cp.tile([NC, 2], F16, tag="cf2h")
            nc.vector.tensor_copy(cf2_h, cf2_s)

            x_sb = [xpool.tile([C_IN + 1, T], F16, tag="x", name=f"xsb{i}") for i in range(2)]
            xq_sb = [xqpool.tile([NC, CH * C_IN], F32, tag="xq", name=f"xqsb{i}") for i in range(2)]
            for i in range(2):
                nc.vector.memset(x_sb[i][C_IN : C_IN + 1, :], 1.0)

            xwt_sb = [sm.tile([C_IN + 1, NC], F16, tag=f"xwt{i}", name=f"xwt{i}") for i in range(2)]
            for i in range(2):
                nc.vector.memset(xwt_sb[i][C_IN : C_IN + 1, :], 1.0)

            lqsA = cp.tile([2, 512 * ((GROUPS[0][2] * 128 + 511) // 512)], F32, tag="lqsA")
            lqsB = cp.tile([2, 512 * ((GROUPS[1][2] * 128 + 511) // 512)], F32, tag="lqsB")
            vTr = cp.tile([128, BPC], F32, tag="vTr")
            vTi = cp.tile([128, BPC], F32, tag="vTi")
            vT = cp.tile([128, BPC], F32, tag="vT")
            prod = cp.tile([128, 19 * BPC], F32, tag="prod")
            qrow = cp.tile([1, 19 * BPC], F32, tag="qrow")
            qfT = cp.tile([19, BPC], F32, tag="qfT")
            ssc = cp.tile([BPC, 1], F32, tag="ssc")
            rss = cp.tile([BPC, 1], F32, tag="rss")

            # ===================== classical =====================
            par_sb = [None] * 8
            for b in range(BPC):
                xb = x_sb[b % 2]
                nc.sync.dma_start(out=xb[0:C_IN, :], in_=xs[b, :, :])

                for blk in range(4):
                    hp = ps_h.tile([128, 512], F32, tag="hp")
                    nc.tensor.matmul(hp, wfb_s, xb[:, blk * 512 : (blk + 1) * 512],
                                     start=True, stop=True)
                    th = thpool.tile([128, 512], F16, tag="th")
                    nc.scalar.activation(th, hp, AF.Tanh)
                    sc = ps_s.tile([1, 512], F32, tag="sc")
                    nc.tensor.matmul(sc, aw2_s, th, start=True, stop=True)
                    ssc_t = sm.tile([1, 512], F32, tag="sscb", name="sscb")
                    if blk % 2 == 0:
                        nc.vector.tensor_copy(ssc_t, sc)
                    else:
                        nc.scalar.copy(ssc_t, sc)
                    gg, bb = b // 8, b % 8
                    src = ssc_t.rearrange("p (n k) -> p n k", n=32, k=CH)
                    dst = sc_g[gg][blk * 32 : (blk + 1) * 32, bb * CH : (bb + 1) * CH]
                    nc.sync.dma_start(out=dst, in_=src)

                if b % 8 == 7:
                    gg = b // 8
                    nc.scalar.activation(esc_g[gg], sc_g[gg], AF.Exp)
                    ssum = sm.tile([NC, 8], F32, tag="ssum")
                    nc.vector.tensor_reduce(
                        ssum, esc_g[gg].rearrange("p (n k) -> p n k", n=8, k=CH),
                        AX.X, ALU.add)
                    rsum = sm.tile([NC, 8], F32, tag="rsum")
                    nc.vector.reciprocal(rsum, ssum)
                    for bb in range(8):
                        nc.vector.tensor_scalar_mul(
                            w_g[gg][:, bb * CH : (bb + 1) * CH],
                            esc_g[gg][:, bb * CH : (bb + 1) * CH],
                            rsum[:, bb : bb + 1])

                    for bb in range(8):
                        bfull = gg * 8 + bb
                        xqb = xq_sb[bfull % 2]
                        nc.sync.dma_start(out=xqb, in_=xq[bfull, :, :])
                        # xw[nc, c] = sum_k w[nc,k] * xq[nc, c*16+k]
                        tmpxw = sm.tile([NC, CH * C_IN], F32, tag="tmpxw")
                        wv = rawv(w_g[gg], bb * CH, [[0, C_IN], [1, CH]])
                        nc.vector.tensor_tensor(
                            rawv(tmpxw, 0, [[CH, C_IN], [1, CH]]),
                            rawv(xqb, 0, [[CH, C_IN], [1, CH]]),
                            wv, ALU.mult)
                        xw = sm.tile([NC, C_IN], F32, tag="xw")
                        nc.vector.tensor_reduce(
                            xw, tmpxw.rearrange("p (c k) -> p c k", c=C_IN, k=CH),
                            AX.X, ALU.add)
                        xwt_ps = ps_m.tile([C_IN, NC], F32, tag="m")
                        nc.tensor.transpose(xwt_ps, xw, idn_s)
                        xwt = xwt_sb[bfull % 2]
                        nc.scalar.copy(xwt[0:C_IN, :], xwt_ps)
                        cht = [None, None]
                        for h in range(2):
                            chp = ps_m.tile([128, NC], F32, tag="m")
                            nc.tensor.matmul(chp, ewb_s[:, h * 128 : (h + 1) * 128],
                                             xwt, start=True, stop=True)
                            cht[h] = sm.tile([128, NC], F16, tag=f"cht{h}", name=f"cht{h}")
                            nc.scalar.copy(cht[h], chp)
                        par = ps_t.tile([NC, 60], F32, tag="t")
                        nc.tensor.matmul(par, cht[0], pjw_s[:, 0:60], start=True, stop=False)
                        nc.tensor.matmul(par, cht[1], pjw_s[:, 60:120], start=False, stop=False)
                        nc.tensor.matmul(par, ones_h, pjb_s, start=False, stop=True)
                        par_s = sm.tile([NC, 60], F32, tag=f"pars{bb}", name=f"pars{bb}")
                        nc.scalar.activation(par_s, par, AF.Sigmoid)
                        par_sb[bb] = par_s
                    # sins batched after all sigmoids (one ACT table swap)
                    for bb in range(8):
                        bfull = gg * 8 + bb
                        par_s = par_sb[bb]
                        for g in grp:
                            if g["b0"] <= bfull < g["b0"] + g["NB"]:
                                bl = bfull - g["b0"]
                                NB = g["NB"]
                                nc.scalar.activation(
                                    rawv(g["co"], bl, [[NB, 60]]), par_s,
                                    AF.Sin, bias=float(np.pi / 2), scale=0.5)
                                nc.scalar.activation(
                                    rawv(g["si"], bl, [[NB, 60]]), par_s,
                                    AF.Sin, bias=0.0, scale=0.5)
                                break

            # ===================== quantum stage 1 (batched) =====================
            for g in grp:
                eng, NB = g["eng"], g["NB"]
                eng.memset(g["ST"], 0.0)
                eng.memset(g["ST"][:, 0:NB], 1.0)

            gates = ansatz_gates(2)
            crx_ctr = 0
            for gi, (kind, loc, j) in enumerate(gates):
                ah = (gi // 3) + 1 if (kind != "crx" and gi < 18) else 6
                for g in grp:
                    eng, NB, ST, TMP = g["eng"], g["NB"], g["ST"], g["TMP"]
                    co, si = g["co"], g["si"]
                    if kind != "crx":
                        p = 5 - loc
                        if ah == 6:
                            sv = bv(ST, NB, None, {}, 6, split_b=True)
                            tv = bv(TMP, NB, None, {}, 6, split_b=True)
                            cts = counts_of(sv)
                            eng.tensor_tensor(tv, sv, coeffv(si, j * NB, cts), ALU.mult)
                            eng.tensor_tensor(sv, sv, coeffv(co, j * NB, cts), ALU.mult)
                        else:
                            for ri in (0, 1):
                                sv = bv(ST, NB, ri, {}, ah)
                                tv = bv(TMP, NB, ri, {}, ah)
                                cts = counts_of(sv)
                                eng.tensor_tensor(tv, sv, coeffv(si, j * NB, cts), ALU.mult)
                                eng.tensor_tensor(sv, sv, coeffv(co, j * NB, cts), ALU.mult)
                        for (oR, oK, iR, iK, op) in ROT_ADDS[kind]:
                            ov = bv(ST, NB, oR, {p: oK}, ah)
                            iv = bv(TMP, NB, iR, {p: iK}, ah)
                            eng.tensor_tensor(ov, ov, iv, op)
                    else:
                        wc, wt = loc
                        pc, pt = 5 - wc, 5 - wt
                        sx = g["sexp"][crx_ctr % 2]
                        cx = g["cexp"][crx_ctr % 2]
                        # Pool group expands its own coefficients: a shared ACT
                        # stream head-of-line-blocks the fast group behind the
                        # slow one.
                        if g["eng"] is nc.gpsimd:
                            eng.tensor_copy(rawv(sx, 0, [[NB, 32], [1, NB]]),
                                            coeffv(si, j * NB, [32, NB]))
                            eng.tensor_copy(rawv(cx, 0, [[NB, 32], [1, NB]]),
                                            coeffv(co, j * NB, [32, NB]))
                        else:
                            nc.scalar.copy(rawv(sx, 0, [[NB, 32], [1, NB]]),
                                           coeffv(si, j * NB, [32, NB]))
                            nc.scalar.copy(rawv(cx, 0, [[NB, 32], [1, NB]]),
                                           coeffv(co, j * NB, [32, NB]))
                        for ri in (0, 1):
                            sv = bv(ST, NB, ri, {pc: 1})
                            tv = bv(TMP, NB, ri, {pc: 1})
                            cts = counts_of(sv)
                            eng.tensor_tensor(tv, sv, coeffv(sx, 0, cts), ALU.mult)
                        for ri in (0, 1):
                            sv = bv(ST, NB, ri, {pc: 1})
                            cts = counts_of(sv)
                            eng.tensor_tensor(sv, sv, coeffv(cx, 0, cts), ALU.mult)
                        for kt in (0, 1):
                            ov = bv(ST, NB, 0, {pc: 1, pt: kt})
                            iv = bv(TMP, NB, 1, {pc: 1, pt: 1 - kt})
                            eng.tensor_tensor(ov, ov, iv, ALU.add)
                            ov = bv(ST, NB, 1, {pc: 1, pt: kt})
                            iv = bv(TMP, NB, 0, {pc: 1, pt: 1 - kt})
                            eng.tensor_tensor(ov, ov, iv, ALU.subtract)
                if kind == "crx":
                    crx_ctr += 1

            # ===================== LCU (one matmul per 512 chunk) ===============
            for g, lqs in ((grp[0], lqsA), (grp[1], lqsB)):
                NB = g["NB"]
                cfl = cf2_h if g["dt"] == F16 else cf2_s
                w = 128 * NB
                c0 = 0
                while c0 < w:
                    cw = min(512, w - c0)
                    lp = ps_s.tile([2, 512], F32, tag="sc")
                    nc.tensor.matmul(lp[:, 0:cw], cfl, g["ST"][:, c0 : c0 + cw],
                                     start=True, stop=True)
                    nc.scalar.copy(lqs[:, c0 : c0 + cw], lp[:, 0:cw])
                    c0 += cw

            # scatter rows into vTr / vTi (ri-swapped)
            for g, lqs in ((grp[0], lqsA), (grp[1], lqsB)):
                NB, b0 = g["NB"], g["b0"]
                nc.sync.dma_start(
                    out=vTr[:, b0 : b0 + NB],
                    in_=rawv(lqs[0:1, 0:1], 0, [[NB, 128], [1, NB]]))
                nc.sync.dma_start(
                    out=vTi[0:64, b0 : b0 + NB],
                    in_=rawv(lqs[1:2, 0:1], 64 * NB, [[NB, 64], [1, NB]]))
                nc.sync.dma_start(
                    out=vTi[64:128, b0 : b0 + NB],
                    in_=rawv(lqs[1:2, 0:1], 0, [[NB, 64], [1, NB]]))
            nc.vector.tensor_tensor(vT[0:64, :], vTr[0:64, :], vTi[0:64, :],
                                    ALU.subtract)
            nc.vector.tensor_tensor(vT[64:128, :], vTr[64:128, :], vTi[64:128, :],
                                    ALU.add)

            # ============== 19 quadratic forms  qfeat_i = v^T M_i v ==============
            t19 = ps_m.tile([128, 19 * BPC], F32, tag="m")
            for i in range(19):
                nc.tensor.matmul(t19[:, i * BPC : (i + 1) * BPC],
                                 mt_s[:, i * 128 : (i + 1) * 128], vT,
                                 start=True, stop=True)
            nc.vector.tensor_tensor(
                rawv(prod, 0, [[BPC, 19], [1, BPC]]),
                rawv(t19, 0, [[BPC, 19], [1, BPC]]),
                rawv(vT, 0, [[0, 19], [1, BPC]]), ALU.mult)
            qp_ps = ps_s.tile([1, 512], F32, tag="sc")
            nc.tensor.matmul(qp_ps[:, 0 : 19 * BPC], ones_col, prod,
                             start=True, stop=True)
            nc.scalar.copy(qrow, qp_ps[:, 0 : 19 * BPC])
            nc.sync.dma_start(out=qfT,
                              in_=qrow.rearrange("p (a b) -> p a b", a=19, b=BPC))
            nc.sync.dma_start(out=ssc,
                              in_=qfT[18:19, :].rearrange("p (a b) -> p a b", a=BPC, b=1))
            nc.vector.reciprocal(rss, ssc)

            # out head: o1 = (qfT^T @ owb) / ss   (row18 = ss pairs with out_b row)
            o1_ps = ps_t.tile([BPC, D], F32, tag="t")
            nc.tensor.matmul(o1_ps, qfT, owb_s, start=True, stop=True)
            o1 = sm.tile([BPC, D], F32, tag="o1")
            nc.vector.tensor_scalar_mul(o1, o1_ps, rss[:, 0:1])

            # LayerNorm
            stats = sm.tile([BPC, 6], F32, tag="stats")
            nc.vector.bn_stats(stats, o1)
            mv = sm.tile([BPC, 2], F32, tag="mv")
            nc.vector.bn_aggr(mv, stats)
            sdv = sm.tile([BPC, 1], F32, tag="sdv")
            nc.scalar.activation(sdv, mv[:, 1:2], AF.Sqrt, bias=1e-5)
            rstd = sm.tile([BPC, 1], F32, tag="rstd")
            nc.vector.reciprocal(rstd, sdv)
            ln1 = sm.tile([BPC, D], F32, tag="ln1")
            nc.vector.tensor_scalar(ln1, o1, mv[:, 0:1], rstd,
                                    ALU.subtract, ALU.mult)
            ln2 = sm.tile([BPC, D], F32, tag="ln2")
            nc.vector.tensor_tensor(ln2, ln1, lng_s, ALU.mult)
            nc.vector.tensor_tensor(ln2, ln2, lnb_s, ALU.add)

            # classifier
            lnT = [None, None]
            for h in range(2):
                lnT_ps = ps_m.tile([128, BPC], F32, tag="m")
                nc.tensor.transpose(lnT_ps, ln2[:, h * 128 : (h + 1) * 128],
                                    idn_s[0:BPC, 0:BPC])
                lnT[h] = sm.tile([128, BPC], F32, tag=f"lnT{h}", name=f"lnT{h}")
                nc.scalar.copy(lnT[h], lnT_ps)
            h2p = ps_t.tile([BPC, D], F32, tag="t")
            nc.tensor.matmul(h2p, lnT[0], cw1_s[:, 0:D], start=True, stop=False)
            nc.tensor.matmul(h2p, lnT[1], cw1_s[:, D : 2 * D], start=False, stop=False)
            nc.tensor.matmul(h2p, ones[:, 0:BPC], cb1_s, start=False, stop=True)
            h2 = sm.tile([BPC, D], F32, tag="h2")
            nc.scalar.activation(h2, h2p, AF.Relu)

            h2T = [None, None]
            for h in range(2):
                h2T_ps = ps_m.tile([128, BPC], F32, tag="m")
                nc.tensor.transpose(h2T_ps, h2[:, h * 128 : (h + 1) * 128],
                                    idn_s[0:BPC, 0:BPC])
                h2T[h] = sm.tile([128, BPC], F32, tag=f"h2T{h}", name=f"h2T{h}")
                nc.scalar.copy(h2T[h], h2T_ps)
            lg = ps_t.tile([BPC, 2], F32, tag="t")
            nc.tensor.matmul(lg, h2T[0], cw2_s[:, 0:2], start=True, stop=False)
            nc.tensor.matmul(lg, h2T[1], cw2_s[:, 2:4], start=False, stop=False)
            nc.tensor.matmul(lg, ones[:, 0:BPC], cb2_s, start=False, stop=True)
            lgs = sm.tile([BPC, 2], F32, tag="lgs")
            nc.vector.tensor_copy(lgs, lg)
            nc.sync.dma_start(out=out[:, :], in_=lgs)

    if split_waits:
        _split_multi_waits(nc)
    return nc


_NC_CACHE = {}


def _get_program():
    if "nc" not in _NC_CACHE:
        _NC_CACHE["nc"] = build_program()
    return _NC_CACHE["nc"]


# ----------------------------------------------------------------- host side
def _host_qff_matrices(qff_params, out_w):
    """19 symmetric 128x128 real matrices M_i = Ureal^T P_real_i Ureal."""
    qp = np.asarray(qff_params, np.float64)
    U = np.eye(DIM, dtype=np.complex128)

    def gate_1q(g2, wire):
        return np.kron(np.kron(np.eye(1 << wire), g2),
                       np.eye(1 << (NQ - 1 - wire)))

    def rx(t):
        c, s = np.cos(t / 2), np.sin(t / 2)
        return np.array([[c, -1j * s], [-1j * s, c]])

    def ry(t):
        c, s = np.cos(t / 2), np.sin(t / 2)
        return np.array([[c, -s], [s, c]])

    def rz(t):
        e = np.exp(-0.5j * t)
        return np.array([[e, 0], [0, np.conj(e)]])

    def crx_full(t, ctrl, tgt):
        G = np.eye(DIM, dtype=np.complex128)
        cb, tb = 5 - ctrl, 5 - tgt
        c, s = np.cos(t / 2), np.sin(t / 2)
        for a in range(DIM):
            if (a >> cb) & 1:
                G[a, a] = c
                G[a, a ^ (1 << tb)] = -1j * s
        return G

    for (kind, loc, j) in ansatz_gates(1):
        th = qp[j]
        if kind == "crx":
            G = crx_full(th, loc[0], loc[1])
        else:
            g2 = {"rx": rx, "ry": ry, "rz": rz}[kind](th)
            G = gate_1q(g2, loc)
        U = G @ U

    PX = np.array([[0, 1], [1, 0]], np.complex128)
    PY = np.array([[0, -1j], [1j, 0]], np.complex128)
    PZ = np.array([[1, 0], [0, -1]], np.complex128)

    mats = []
    for P in (PX, PY, PZ):
        for i in range(NQ):
            Pi = np.kron(np.kron(np.eye(1 << i), P), np.eye(1 << (NQ - 1 - i)))
            M = U.conj().T @ Pi @ U
            A, B = M.real, M.imag
            mats.append(np.block([[A, -B], [B, A]]))
    mats.append(np.eye(2 * DIM))
    MT = np.stack(mats, 0)  # [19, 128, 128]
    # lhsT[k, m] = M[m, k]; M symmetric -> store as-is
    return np.ascontiguousarray(
        MT.transpose(1, 0, 2).reshape(128, 19 * 128)).astype(np.float32)


def host_prep(inputs):
    f32 = np.float32
    x = np.asarray(inputs["x"], f32)
    emb_w = np.asarray(inputs["emb_w"], np.float64)
    emb_b = np.asarray(inputs["emb_b"], np.float64)
    att_w1 = np.asarray(inputs["att_w1"], np.float64)
    att_b1 = np.asarray(inputs["att_b1"], np.float64)

    f16 = np.float16
    wfold = (emb_w @ att_w1).astype(f16)
    bfold = (emb_b @ att_w1 + att_b1).astype(f16)
    wfb = np.concatenate([wfold, bfold[None, :]], 0)

    ewb = np.concatenate([emb_w.astype(f16), emb_b.astype(f16)[None, :]], 0)

    pw = np.asarray(inputs["proj_w"], f32)
    pjw = np.concatenate([pw[0:128, :], pw[128:256, :]], 1).astype(f16)

    cr = np.asarray(inputs["mix_re"], np.float64)
    ci = np.asarray(inputs["mix_im"], np.float64)
    den = np.sqrt(cr * cr + ci * ci).sum() + 1e-8
    cf2 = np.stack([cr / den, ci / den], 1).astype(f32)

    mt_m = _host_qff_matrices(inputs["qff_params"], inputs["out_w"])

    owb = np.concatenate(
        [np.asarray(inputs["out_w"], f32), np.asarray(inputs["out_b"], f32)[None, :]], 0)
    lng = np.broadcast_to(np.asarray(inputs["ln_g"], f32), (BPC, D)).copy()
    lnb = np.broadcast_to(np.asarray(inputs["ln_b"], f32), (BPC, D)).copy()
    w1 = np.asarray(inputs["cls_w1"], f32)
    cw1 = np.concatenate([w1[0:128, :], w1[128:256, :]], 1)
    cb1 = np.asarray(inputs["cls_b1"], f32)[None, :]
    w2 = np.asarray(inputs["cls_w2"], f32)
    cw2 = np.concatenate([w2[0:128, :], w2[128:256, :]], 1)
    cb2 = np.asarray(inputs["cls_b2"], f32)[None, :]
    idn = np.eye(128, dtype=f32)
    pjb = np.asarray(inputs["proj_b"], f16)[None, :]

    shared = dict(wfb=wfb, aw2=np.asarray(inputs["att_w2"], f16), ewb=ewb,
                  pjw=pjw, pjb=pjb, cf2=cf2, mt=mt_m, owb=owb, lng=lng,
                  lnb=lnb, cw1=cw1, cb1=cb1, cw2=cw2, cb2=cb2, idn=idn)

    in_maps = []
    for c in range(N_CORES):
        xc = x[c * BPC : (c + 1) * BPC]
        # xq[b, nc, cc*16+k] = x[b, cc, nc*16+k]  (c-major)
        xq_c = np.ascontiguousarray(
            xc.reshape(BPC, C_IN, NC, CH).transpose(0, 2, 1, 3).reshape(
                BPC, NC, C_IN * CH))
        m = dict(shared)
        m["xs"] = np.ascontiguousarray(xc).astype(np.float16)
        m["xq"] = xq_c
        in_maps.append(m)
    return in_maps


def kernel(**inputs):
    nc = _get_program()
    in_maps = host_prep(inputs)
    res = run_bass_kernel_spmd(nc, in_maps, core_ids=list(range(N_CORES)))
    outs = [res.results[c]["out"] for c in range(N_CORES)]
    return np.concatenate(outs, 0).astype(np.float32)


if __name__ == "__main__":
    nc = build_program()
    print("program built ok")


# revision 14
# speedup vs baseline: 2.5660x; 1.1623x over previous
"""Trainium2 Bass kernel for nn_ClassicalQuantumAttention (batched rewrite).

Data-parallel over batch: 128 batch elems -> 16 per NeuronCore x 8 cores.

Quantum stage is BATCHED: per engine-group g with NB batch elems, one state
tile ST [128 nc, 128*NB] with free index f = q*NB + b  (q = ri*64 + a,
ri = re/im, a = 6-bit amplitude).  Each gate is ~6 big tensor_tensor ops
over all NB elems at once; per-(nc,b) cos/sin coefficients are read via
stride-0 broadcast views of [128, 60*NB] coefficient tiles.  Groups run on
different engines (DVE / Pool) as independent pipelines.

qff ansatz + expvals are folded on host into 19 symmetric 128x128 matrices
M_i = Ureal^T P_i Ureal (M_18 = I for the squared norm): qfeat_i[b] =
v_b^T M_i v_b via 19 PE matmuls + one elementwise mul + a ones-matmul
partition reduction.  LCU mixing is one K=128 matmul per 512-wide chunk.
"""

import numpy as np
import sys

for _p in ("/opt/trn_rl_repo",):
    if _p not in sys.path:
        sys.path.insert(0, _p)

import concourse.bass as bass
import concourse.tile as tile
from concourse import mybir
from concourse.bass_utils import run_bass_kernel_spmd

F32 = mybir.dt.float32
F16 = mybir.dt.float16
ALU = mybir.AluOpType
AF = mybir.ActivationFunctionType
AX = mybir.AxisListType

N_CORES = 8
B_TOT = 128
BPC = B_TOT // N_CORES  # 16
C_IN = 64
T = 2048
D = 256
CH = 16
NC = T // CH  # 128
NQ = 6
DIM = 64

# (engine_attr, b_start, NB, state_dtype) — fp16 state enables the DVE 2x
# tensor_tensor mode; Pool stays fp32 (Q7 software path).
GROUPS = [("vector", 0, 14, F16), ("gpsimd", 14, 2, F16)]


def ansatz_gates(n_layers):
    gates = []
    idx = 0
    for _ in range(n_layers):
        for i in range(NQ):
            gates.append(("rx", i, idx))
            gates.append(("ry", i, idx + 1))
            gates.append(("rz", i, idx + 2))
            idx += 3
        for i in range(NQ):
            gates.append(("crx", (i, (i + 1) % NQ), idx))
            idx += 1
        for i in range(NQ - 1, -1, -1):
            gates.append(("crx", (i, (i - 1) % NQ), idx))
            idx += 1
    return gates


# --------------------------------------------------------------- AP helpers
def bv(t, NB, ri, fixed, ah=6, split_b=False):
    """Batched state view of t [128, 128*NB], f = q*NB + b, q = ri*64 + a.

    ri: 0/1 or None (both halves; requires full contiguity).
    fixed: {amp_bit: 0/1}.  ah: active high bits (L1 sparsity support).
    split_b: represent a single full run as [[NB, n],[1, NB]] for coefficient
    shape-matching.
    """
    part = t.ap[0]
    off = t.offset
    lo_active = 6 - ah
    dims = []  # inner-first
    run = [1, NB]
    for p in range(6):
        w = NB * (1 << p)
        if p in fixed:
            off += fixed[p] * w
            if run is not None:
                dims.append(run)
                run = None
        elif p < lo_active:
            if run is not None:
                dims.append(run)
                run = None
        else:
            if run is not None and run[0] * run[1] == w:
                run[1] *= 2
            elif run is not None:
                dims.append(run)
                run = [w, 2]
            else:
                run = [w, 2]
    if ri is None:
        w = NB * 64
        assert run is not None and run[0] * run[1] == w and not dims, "ri-merge"
        run[1] *= 2
    else:
        off += ri * NB * 64
    if run is not None:
        dims.append(run)
    dims = dims[::-1]
    if split_b and len(dims) == 1 and dims[0][0] == 1:
        n = dims[0][1] // NB
        dims = [[NB, n], [1, NB]]
    assert 1 <= len(dims) <= 2, f"bv dims {dims}"
    return bass.AP(tensor=t.tensor, offset=off, ap=[list(part)] + dims)


def counts_of(ap):
    return [d[1] for d in ap.ap[1:]]


def coeffv(t, elem_off, counts):
    """Stride-0 broadcast view of coefficient tile t at elem_off matching
    counts ([outer, inner] -> [[0, outer], [1, inner]])."""
    if len(counts) == 2:
        dims = [[0, counts[0]], [1, counts[1]]]
    else:
        dims = [[1, counts[0]]]
    return bass.AP(tensor=t.tensor, offset=t.offset + elem_off,
                   ap=[list(t.ap[0])] + dims)


def rawv(t, elem_off, dims):
    return bass.AP(tensor=t.tensor, offset=t.offset + elem_off,
                   ap=[list(t.ap[0])] + dims)


# rotation add tables: (out_ri, out_k, in_ri, in_k, op)
ROT_ADDS = {
    "rx": [(0, 0, 1, 1, ALU.add), (0, 1, 1, 0, ALU.add),
           (1, 0, 0, 1, ALU.subtract), (1, 1, 0, 0, ALU.subtract)],
    "ry": [(0, 0, 0, 1, ALU.subtract), (0, 1, 0, 0, ALU.add),
           (1, 0, 1, 1, ALU.subtract), (1, 1, 1, 0, ALU.add)],
    "rz": [(0, 0, 1, 0, ALU.add), (1, 0, 0, 0, ALU.subtract),
           (0, 1, 1, 1, ALU.subtract), (1, 1, 0, 1, ALU.add)],
}


def _split_multi_waits(nc):
    """Walrus build allows at most ONE sync-wait per instruction; hoist
    extra waits onto same-engine NoOps."""
    ctr = [0]
    for f in nc.m.functions:
        for b in f.blocks:
            new = []
            for inst in b.instructions:
                si = inst.sync_info
                if si is not None and len(si.on_wait) > 1:
                    waits = list(si.on_wait)
                    for w in waits[:-1]:
                        ctr[0] += 1
                        nop = mybir.InstNoOp(
                            name=f"wsplit-{ctr[0]}",
                            ins=[], outs=[],
                            engine=inst.engine,
                            sync_info=mybir.SyncInfo(on_wait=[w], on_update=[]),
                        )
                        new.append(nop)
                    inst.sync_info = mybir.SyncInfo(
                        on_wait=[waits[-1]], on_update=list(si.on_update)
                    )
                new.append(inst)
            b.instructions = new


# ----------------------------------------------------------------- program
def build_program(split_waits=True):
    nc = bass.Bass()

    for vconst in (float(np.pi / 2), 1e-5):
        t = nc.alloc_sbuf_tensor(f"const-f32-{vconst}", [128, 1], F32)
        nc.gpsimd.memset(t.ap(), vconst)
        nc.const_aps.aps[(F32, vconst)] = t.ap()
    nc.all_engine_barrier()

    xs = nc.declare_dram_parameter("xs", [BPC, C_IN, T], F16, isOutput=False)
    xq = nc.declare_dram_parameter("xq", [BPC, NC, CH * C_IN], F32, isOutput=False)
    wfb = nc.declare_dram_parameter("wfb", [C_IN + 1, 128], F16, isOutput=False)
    aw2 = nc.declare_dram_parameter("aw2", [128, 1], F16, isOutput=False)
    ewb = nc.declare_dram_parameter("ewb", [C_IN + 1, D], F16, isOutput=False)
    pjw = nc.declare_dram_parameter("pjw", [128, 120], F16, isOutput=False)
    pjb = nc.declare_dram_parameter("pjb", [1, 60], F16, isOutput=False)
    cf2 = nc.declare_dram_parameter("cf2", [NC, 2], F32, isOutput=False)
    mt = nc.declare_dram_parameter("mt", [128, 19 * 128], F16, isOutput=False)
    owb = nc.declare_dram_parameter("owb", [19, D], F32, isOutput=False)
    lng = nc.declare_dram_parameter("lng", [BPC, D], F32, isOutput=False)
    lnb = nc.declare_dram_parameter("lnb", [BPC, D], F32, isOutput=False)
    cw1 = nc.declare_dram_parameter("cw1", [128, 2 * D], F32, isOutput=False)
    cb1 = nc.declare_dram_parameter("cb1", [1, D], F32, isOutput=False)
    cw2 = nc.declare_dram_parameter("cw2", [128, 4], F32, isOutput=False)
    cb2 = nc.declare_dram_parameter("cb2", [1, 2], F32, isOutput=False)
    idn = nc.declare_dram_parameter("idn", [128, 128], F32, isOutput=False)
    out = nc.declare_dram_parameter("out", [BPC, 2], F32, isOutput=True)

    with tile.TileContext(nc) as tc:
        with (
            tc.tile_pool(name="const", bufs=1) as cp,
            tc.tile_pool(name="xbuf", bufs=2) as xpool,
            tc.tile_pool(name="xqbuf", bufs=2) as xqpool,
            tc.tile_pool(name="tanh", bufs=2) as thpool,
            tc.tile_pool(name="small", bufs=4) as sm,
            tc.tile_pool(name="ps_h", bufs=2, space="PSUM") as ps_h,
            tc.tile_pool(name="ps_s", bufs=2, space="PSUM") as ps_s,
            tc.tile_pool(name="ps_m", bufs=2, space="PSUM") as ps_m,
            tc.tile_pool(name="ps_t", bufs=2, space="PSUM") as ps_t,
        ):
            def cload(name, dram, shape, dt=F32):
                t = cp.tile(shape, dt, tag=name, name=name)
                nc.sync.dma_start(out=t, in_=dram[:, :])
                return t

            wfb_s = cload("wfb", wfb, [C_IN + 1, 128], F16)
            aw2_s = cload("aw2", aw2, [128, 1], F16)
            ewb_s = cload("ewb", ewb, [C_IN + 1, D], F16)
            pjw_s = cload("pjw", pjw, [128, 120], F16)
            pjb_s = cload("pjb", pjb, [1, 60], F16)
            cf2_s = cload("cf2", cf2, [NC, 2])
            mt_s = cload("mt", mt, [128, 19 * 128], F16)
            owb_s = cload("owb", owb, [19, D])
            lng_s = cload("lng", lng, [BPC, D])
            lnb_s = cload("lnb", lnb, [BPC, D])
            cw1_s = cload("cw1", cw1, [128, 2 * D])
            cb1_s = cload("cb1", cb1, [1, D])
            cw2_s = cload("cw2", cw2, [128, 4])
            cb2_s = cload("cb2", cb2, [1, 2])
            idn_s = cload("idn", idn, [128, 128])

            ones = cp.tile([1, 128], F32, tag="ones")
            nc.vector.memset(ones, 1.0)
            ones_col = cp.tile([128, 1], F32, tag="ones_col")
            nc.vector.memset(ones_col, 1.0)
            ones_h = cp.tile([1, 128], F16, tag="ones_h")
            nc.vector.memset(ones_h, 1.0)

            sc_g = [cp.tile([NC, 8 * CH], F32, tag=f"scg{g}", name=f"scg{g}") for g in range(2)]
            esc_g = [cp.tile([NC, 8 * CH], F32, tag=f"escg{g}", name=f"escg{g}") for g in range(2)]
            w_g = [cp.tile([NC, 8 * CH], F32, tag=f"wg{g}", name=f"wg{g}") for g in range(2)]

            # per-engine-group quantum tiles
            grp = []
            for gi_, (engname, b0, NB, sdt) in enumerate(GROUPS):
                g = dict(
                    eng=getattr(nc, engname), b0=b0, NB=NB, dt=sdt,
                    ST=cp.tile([128, 128 * NB], sdt, tag=f"ST{gi_}", name=f"ST{gi_}"),
                    TMP=cp.tile([128, 128 * NB], sdt, tag=f"TMP{gi_}", name=f"TMP{gi_}"),
                    co=cp.tile([128, 60 * NB], sdt, tag=f"co{gi_}", name=f"co{gi_}"),
                    si=cp.tile([128, 60 * NB], sdt, tag=f"si{gi_}", name=f"si{gi_}"),
                    sexp=[cp.tile([128, 32 * NB], sdt, tag=f"sx{gi_}{k}", name=f"sx{gi_}{k}") for k in range(2)],
                    cexp=[cp.tile([128, 32 * NB], sdt, tag=f"cx{gi_}{k}", name=f"cx{gi_}{k}") for k in range(2)],
                )
                grp.append(g)
            cf2_h = cp.tile([NC, 2], F16, tag="cf2h")
            nc.vector.tensor_copy(cf2_h, cf2_s)

            x_sb = [xpool.tile([C_IN + 1, T], F16, tag="x", name=f"xsb{i}") for i in range(2)]
            xq_sb = [xqpool.tile([NC, CH * C_IN], F32, tag="xq", name=f"xqsb{i}") for i in range(2)]
            for i in range(2):
                nc.vector.memset(x_sb[i][C_IN : C_IN + 1, :], 1.0)

            xwt_sb = [sm.tile([C_IN + 1, NC], F16, tag=f"xwt{i}", name=f"xwt{i}") for i in range(2)]
            for i in range(2):
                nc.vector.memset(xwt_sb[i][C_IN : C_IN + 1, :], 1.0)

            lqsA = cp.tile([2, 512 * ((GROUPS[0][2] * 128 + 511) // 512)], F32, tag="lqsA")
            lqsB = cp.tile([2, 512 * ((GROUPS[1][2] * 128 + 511) // 512)], F32, tag="lqsB")
            vTr = cp.tile([128, BPC], F32, tag="vTr")
            vTi = cp.tile([128, BPC], F32, tag="vTi")
            vT = cp.tile([128, BPC], F32, tag="vT")
            vT_h = cp.tile([128, BPC], F16, tag="vTh")
            prod = cp.tile([128, 19 * BPC], F32, tag="prod")
            qrow = cp.tile([1, 19 * BPC], F32, tag="qrow")
            qfT = cp.tile([19, BPC], F32, tag="qfT")
            ssc = cp.tile([BPC, 1], F32, tag="ssc")
            rss = cp.tile([BPC, 1], F32, tag="rss")

            # ===================== classical =====================
            par_sb = [None] * 8
            for b in range(BPC):
                xb = x_sb[b % 2]
                nc.sync.dma_start(out=xb[0:C_IN, :], in_=xs[b, :, :])

                for blk in range(4):
                    hp = ps_h.tile([128, 512], F32, tag="hp")
                    nc.tensor.matmul(hp, wfb_s, xb[:, blk * 512 : (blk + 1) * 512],
                                     start=True, stop=True)
                    th = thpool.tile([128, 512], F16, tag="th")
                    nc.scalar.activation(th, hp, AF.Tanh)
                    sc = ps_s.tile([1, 512], F32, tag="sc")
                    nc.tensor.matmul(sc, aw2_s, th, start=True, stop=True)
                    ssc_t = sm.tile([1, 512], F32, tag="sscb", name="sscb")
                    nc.vector.tensor_copy(ssc_t, sc)
                    gg, bb = b // 8, b % 8
                    src = ssc_t.rearrange("p (n k) -> p n k", n=32, k=CH)
                    dst = sc_g[gg][blk * 32 : (blk + 1) * 32, bb * CH : (bb + 1) * CH]
                    nc.sync.dma_start(out=dst, in_=src)

                if b % 8 == 7:
                    gg = b // 8
                    nc.scalar.activation(esc_g[gg], sc_g[gg], AF.Exp)
                    ssum = sm.tile([NC, 8], F32, tag="ssum")
                    nc.vector.tensor_reduce(
                        ssum, esc_g[gg].rearrange("p (n k) -> p n k", n=8, k=CH),
                        AX.X, ALU.add)
                    rsum = sm.tile([NC, 8], F32, tag="rsum")
                    nc.vector.reciprocal(rsum, ssum)
                    for bb in range(8):
                        nc.vector.tensor_scalar_mul(
                            w_g[gg][:, bb * CH : (bb + 1) * CH],
                            esc_g[gg][:, bb * CH : (bb + 1) * CH],
                            rsum[:, bb : bb + 1])

                    for bb in range(8):
                        bfull = gg * 8 + bb
                        xqb = xq_sb[bfull % 2]
                        nc.sync.dma_start(out=xqb, in_=xq[bfull, :, :])
                        # xw[nc, c] = sum_k w[nc,k] * xq[nc, c*16+k]
                        tmpxw = sm.tile([NC, CH * C_IN], F32, tag="tmpxw")
                        wv = rawv(w_g[gg], bb * CH, [[0, C_IN], [1, CH]])
                        nc.vector.tensor_tensor(
                            rawv(tmpxw, 0, [[CH, C_IN], [1, CH]]),
                            rawv(xqb, 0, [[CH, C_IN], [1, CH]]),
                            wv, ALU.mult)
                        xw = sm.tile([NC, C_IN], F32, tag="xw")
                        nc.vector.tensor_reduce(
                            xw, tmpxw.rearrange("p (c k) -> p c k", c=C_IN, k=CH),
                            AX.X, ALU.add)
                        xwt_ps = ps_m.tile([C_IN, NC], F32, tag="m")
                        nc.tensor.transpose(xwt_ps, xw, idn_s)
                        xwt = xwt_sb[bfull % 2]
                        nc.scalar.copy(xwt[0:C_IN, :], xwt_ps)
                        cht = [None, None]
                        for h in range(2):
                            chp = ps_m.tile([128, NC], F32, tag="m")
                            nc.tensor.matmul(chp, ewb_s[:, h * 128 : (h + 1) * 128],
                                             xwt, start=True, stop=True)
                            cht[h] = sm.tile([128, NC], F16, tag=f"cht{h}", name=f"cht{h}")
                            nc.vector.tensor_copy(cht[h], chp)
                        par = ps_t.tile([NC, 60], F32, tag="t")
                        nc.tensor.matmul(par, cht[0], pjw_s[:, 0:60], start=True, stop=False)
                        nc.tensor.matmul(par, cht[1], pjw_s[:, 60:120], start=False, stop=False)
                        nc.tensor.matmul(par, ones_h, pjb_s, start=False, stop=True)
                        par_s = sm.tile([NC, 60], F32, tag=f"pars{bb}", name=f"pars{bb}")
                        nc.scalar.activation(par_s, par, AF.Sigmoid)
                        par_sb[bb] = par_s
                    # sins batched after all sigmoids (one ACT table swap)
                    for bb in range(8):
                        bfull = gg * 8 + bb
                        par_s = par_sb[bb]
                        for g in grp:
                            if g["b0"] <= bfull < g["b0"] + g["NB"]:
                                bl = bfull - g["b0"]
                                NB = g["NB"]
                                nc.scalar.activation(
                                    rawv(g["co"], bl, [[NB, 60]]), par_s,
                                    AF.Sin, bias=float(np.pi / 2), scale=0.5)
                                nc.scalar.activation(
                                    rawv(g["si"], bl, [[NB, 60]]), par_s,
                                    AF.Sin, bias=0.0, scale=0.5)
                                break

            # ===================== quantum stage 1 (batched) =====================
            for g in grp:
                eng, NB = g["eng"], g["NB"]
                eng.memset(g["ST"], 0.0)
                eng.memset(g["ST"][:, 0:NB], 1.0)

            gates = ansatz_gates(2)
            crx_ctr = 0
            for gi, (kind, loc, j) in enumerate(gates):
                ah = (gi // 3) + 1 if (kind != "crx" and gi < 18) else 6
                for g in grp:
                    eng, NB, ST, TMP = g["eng"], g["NB"], g["ST"], g["TMP"]
                    co, si = g["co"], g["si"]
                    if kind != "crx":
                        p = 5 - loc
                        if ah == 6:
                            sv = bv(ST, NB, None, {}, 6, split_b=True)
                            tv = bv(TMP, NB, None, {}, 6, split_b=True)
                            cts = counts_of(sv)
                            eng.tensor_tensor(tv, sv, coeffv(si, j * NB, cts), ALU.mult)
                            eng.tensor_tensor(sv, sv, coeffv(co, j * NB, cts), ALU.mult)
                        else:
                            for ri in (0, 1):
                                sv = bv(ST, NB, ri, {}, ah)
                                tv = bv(TMP, NB, ri, {}, ah)
                                cts = counts_of(sv)
                                eng.tensor_tensor(tv, sv, coeffv(si, j * NB, cts), ALU.mult)
                                eng.tensor_tensor(sv, sv, coeffv(co, j * NB, cts), ALU.mult)
                        for (oR, oK, iR, iK, op) in ROT_ADDS[kind]:
                            ov = bv(ST, NB, oR, {p: oK}, ah)
                            iv = bv(TMP, NB, iR, {p: iK}, ah)
                            eng.tensor_tensor(ov, ov, iv, op)
                    else:
                        wc, wt = loc
                        pc, pt = 5 - wc, 5 - wt
                        sx = g["sexp"][crx_ctr % 2]
                        cx = g["cexp"][crx_ctr % 2]
                        # Pool group expands its own coefficients: a shared ACT
                        # stream head-of-line-blocks the fast group behind the
                        # slow one.
                        if g["eng"] is nc.gpsimd:
                            eng.tensor_copy(rawv(sx, 0, [[NB, 32], [1, NB]]),
                                            coeffv(si, j * NB, [32, NB]))
                            eng.tensor_copy(rawv(cx, 0, [[NB, 32], [1, NB]]),
                                            coeffv(co, j * NB, [32, NB]))
                        else:
                            nc.scalar.copy(rawv(sx, 0, [[NB, 32], [1, NB]]),
                                           coeffv(si, j * NB, [32, NB]))
                            nc.scalar.copy(rawv(cx, 0, [[NB, 32], [1, NB]]),
                                           coeffv(co, j * NB, [32, NB]))
                        for ri in (0, 1):
                            sv = bv(ST, NB, ri, {pc: 1})
                            tv = bv(TMP, NB, ri, {pc: 1})
                            cts = counts_of(sv)
                            eng.tensor_tensor(tv, sv, coeffv(sx, 0, cts), ALU.mult)
                        for ri in (0, 1):
                            sv = bv(ST, NB, ri, {pc: 1})
                            cts = counts_of(sv)
                            eng.tensor_tensor(sv, sv, coeffv(cx, 0, cts), ALU.mult)
                        for kt in (0, 1):
                            ov = bv(ST, NB, 0, {pc: 1, pt: kt})
                            iv = bv(TMP, NB, 1, {pc: 1, pt: 1 - kt})
                            eng.tensor_tensor(ov, ov, iv, ALU.add)
                            ov = bv(ST, NB, 1, {pc: 1, pt: kt})
                            iv = bv(TMP, NB, 0, {pc: 1, pt: 1 - kt})
                            eng.tensor_tensor(ov, ov, iv, ALU.subtract)
                if kind == "crx":
                    crx_ctr += 1

            # ===================== LCU (one matmul per 512 chunk) ===============
            for g, lqs in ((grp[0], lqsA), (grp[1], lqsB)):
                NB = g["NB"]
                cfl = cf2_h if g["dt"] == F16 else cf2_s
                w = 128 * NB
                c0 = 0
                while c0 < w:
                    cw = min(512, w - c0)
                    lp = ps_s.tile([2, 512], F32, tag="sc")
                    nc.tensor.matmul(lp[:, 0:cw], cfl, g["ST"][:, c0 : c0 + cw],
                                     start=True, stop=True)
                    nc.scalar.copy(lqs[:, c0 : c0 + cw], lp[:, 0:cw])
                    c0 += cw

            # scatter rows into vTr / vTi (ri-swapped)
            for g, lqs in ((grp[0], lqsA), (grp[1], lqsB)):
                NB, b0 = g["NB"], g["b0"]
                nc.sync.dma_start(
                    out=vTr[:, b0 : b0 + NB],
                    in_=rawv(lqs[0:1, 0:1], 0, [[NB, 128], [1, NB]]))
                nc.sync.dma_start(
                    out=vTi[0:64, b0 : b0 + NB],
                    in_=rawv(lqs[1:2, 0:1], 64 * NB, [[NB, 64], [1, NB]]))
                nc.sync.dma_start(
                    out=vTi[64:128, b0 : b0 + NB],
                    in_=rawv(lqs[1:2, 0:1], 0, [[NB, 64], [1, NB]]))
            nc.vector.tensor_tensor(vT[0:64, :], vTr[0:64, :], vTi[0:64, :],
                                    ALU.subtract)
            nc.vector.tensor_tensor(vT[64:128, :], vTr[64:128, :], vTi[64:128, :],
                                    ALU.add)

            # ============== 19 quadratic forms  qfeat_i = v^T M_i v ==============
            nc.scalar.copy(vT_h, vT)
            t19 = ps_m.tile([128, 19 * BPC], F32, tag="m")
            for i in range(19):
                nc.tensor.matmul(t19[:, i * BPC : (i + 1) * BPC],
                                 mt_s[:, i * 128 : (i + 1) * 128], vT_h,
                                 start=True, stop=True)
            nc.vector.tensor_tensor(
                rawv(prod, 0, [[BPC, 19], [1, BPC]]),
                rawv(t19, 0, [[BPC, 19], [1, BPC]]),
                rawv(vT, 0, [[0, 19], [1, BPC]]), ALU.mult)
            qp_ps = ps_s.tile([1, 512], F32, tag="sc")
            nc.tensor.matmul(qp_ps[:, 0 : 19 * BPC], ones_col, prod,
                             start=True, stop=True)
            nc.scalar.copy(qrow, qp_ps[:, 0 : 19 * BPC])
            nc.sync.dma_start(out=qfT,
                              in_=qrow.rearrange("p (a b) -> p a b", a=19, b=BPC))
            nc.sync.dma_start(out=ssc,
                              in_=qfT[18:19, :].rearrange("p (a b) -> p a b", a=BPC, b=1))
            nc.vector.reciprocal(rss, ssc)

            # out head: o1 = (qfT^T @ owb) / ss   (row18 = ss pairs with out_b row)
            o1_ps = ps_t.tile([BPC, D], F32, tag="t")
            nc.tensor.matmul(o1_ps, qfT, owb_s, start=True, stop=True)
            o1 = sm.tile([BPC, D], F32, tag="o1")
            nc.vector.tensor_scalar_mul(o1, o1_ps, rss[:, 0:1])

            # LayerNorm
            stats = sm.tile([BPC, 6], F32, tag="stats")
            nc.vector.bn_stats(stats, o1)
            mv = sm.tile([BPC, 2], F32, tag="mv")
            nc.vector.bn_aggr(mv, stats)
            sdv = sm.tile([BPC, 1], F32, tag="sdv")
            nc.scalar.activation(sdv, mv[:, 1:2], AF.Sqrt, bias=1e-5)
            rstd = sm.tile([BPC, 1], F32, tag="rstd")
            nc.vector.reciprocal(rstd, sdv)
            ln1 = sm.tile([BPC, D], F32, tag="ln1")
            nc.vector.tensor_scalar(ln1, o1, mv[:, 0:1], rstd,
                                    ALU.subtract, ALU.mult)
            ln2 = sm.tile([BPC, D], F32, tag="ln2")
            nc.vector.tensor_tensor(ln2, ln1, lng_s, ALU.mult)
            nc.vector.tensor_tensor(ln2, ln2, lnb_s, ALU.add)

            # classifier
            lnT = [None, None]
            for h in range(2):
                lnT_ps = ps_m.tile([128, BPC], F32, tag="m")
                nc.tensor.transpose(lnT_ps, ln2[:, h * 128 : (h + 1) * 128],
                                    idn_s[0:BPC, 0:BPC])
                lnT[h] = sm.tile([128, BPC], F32, tag=f"lnT{h}", name=f"lnT{h}")
                nc.scalar.copy(lnT[h], lnT_ps)
            h2p = ps_t.tile([BPC, D], F32, tag="t")
            nc.tensor.matmul(h2p, lnT[0], cw1_s[:, 0:D], start=True, stop=False)
            nc.tensor.matmul(h2p, lnT[1], cw1_s[:, D : 2 * D], start=False, stop=False)
            nc.tensor.matmul(h2p, ones[:, 0:BPC], cb1_s, start=False, stop=True)
            h2 = sm.tile([BPC, D], F32, tag="h2")
            nc.scalar.activation(h2, h2p, AF.Relu)

            h2T = [None, None]
            for h in range(2):
                h2T_ps = ps_m.tile([128, BPC], F32, tag="m")
                nc.tensor.transpose(h2T_ps, h2[:, h * 128 : (h + 1) * 128],
                                    idn_s[0:BPC, 0:BPC])
                h2T[h] = sm.tile([128, BPC], F32, tag=f"h2T{h}", name=f"h2T{h}")
                nc.scalar.copy(h2T[h], h2T_ps)
            lg = ps_t.tile([BPC, 2], F32, tag="t")
            nc.tensor.matmul(lg, h2T[0], cw2_s[:, 0:2], start=True, stop=False)
            nc.tensor.matmul(lg, h2T[1], cw2_s[:, 2:4], start=False, stop=False)
            nc.tensor.matmul(lg, ones[:, 0:BPC], cb2_s, start=False, stop=True)
            lgs = sm.tile([BPC, 2], F32, tag="lgs")
            nc.vector.tensor_copy(lgs, lg)
            nc.sync.dma_start(out=out[:, :], in_=lgs)

    if split_waits:
        _split_multi_waits(nc)
    return nc


_NC_CACHE = {}


def _get_program():
    if "nc" not in _NC_CACHE:
        _NC_CACHE["nc"] = build_program()
    return _NC_CACHE["nc"]


# ----------------------------------------------------------------- host side
def _host_qff_matrices(qff_params, out_w):
    """19 symmetric 128x128 real matrices M_i = Ureal^T P_real_i Ureal."""
    qp = np.asarray(qff_params, np.float64)
    U = np.eye(DIM, dtype=np.complex128)

    def gate_1q(g2, wire):
        return np.kron(np.kron(np.eye(1 << wire), g2),
                       np.eye(1 << (NQ - 1 - wire)))

    def rx(t):
        c, s = np.cos(t / 2), np.sin(t / 2)
        return np.array([[c, -1j * s], [-1j * s, c]])

    def ry(t):
        c, s = np.cos(t / 2), np.sin(t / 2)
        return np.array([[c, -s], [s, c]])

    def rz(t):
        e = np.exp(-0.5j * t)
        return np.array([[e, 0], [0, np.conj(e)]])

    def crx_full(t, ctrl, tgt):
        G = np.eye(DIM, dtype=np.complex128)
        cb, tb = 5 - ctrl, 5 - tgt
        c, s = np.cos(t / 2), np.sin(t / 2)
        for a in range(DIM):
            if (a >> cb) & 1:
                G[a, a] = c
                G[a, a ^ (1 << tb)] = -1j * s
        return G

    for (kind, loc, j) in ansatz_gates(1):
        th = qp[j]
        if kind == "crx":
            G = crx_full(th, loc[0], loc[1])
        else:
            g2 = {"rx": rx, "ry": ry, "rz": rz}[kind](th)
            G = gate_1q(g2, loc)
        U = G @ U

    PX = np.array([[0, 1], [1, 0]], np.complex128)
    PY = np.array([[0, -1j], [1j, 0]], np.complex128)
    PZ = np.array([[1, 0], [0, -1]], np.complex128)

    mats = []
    for P in (PX, PY, PZ):
        for i in range(NQ):
            Pi = np.kron(np.kron(np.eye(1 << i), P), np.eye(1 << (NQ - 1 - i)))
            M = U.conj().T @ Pi @ U
            A, B = M.real, M.imag
            mats.append(np.block([[A, -B], [B, A]]))
    mats.append(np.eye(2 * DIM))
    MT = np.stack(mats, 0)  # [19, 128, 128]
    # lhsT[k, m] = M[m, k]; M symmetric -> store as-is
    return np.ascontiguousarray(
        MT.transpose(1, 0, 2).reshape(128, 19 * 128)).astype(np.float16)


def host_prep(inputs):
    f32 = np.float32
    x = np.asarray(inputs["x"], f32)
    emb_w = np.asarray(inputs["emb_w"], np.float64)
    emb_b = np.asarray(inputs["emb_b"], np.float64)
    att_w1 = np.asarray(inputs["att_w1"], np.float64)
    att_b1 = np.asarray(inputs["att_b1"], np.float64)

    f16 = np.float16
    wfold = (emb_w @ att_w1).astype(f16)
    bfold = (emb_b @ att_w1 + att_b1).astype(f16)
    wfb = np.concatenate([wfold, bfold[None, :]], 0)

    ewb = np.concatenate([emb_w.astype(f16), emb_b.astype(f16)[None, :]], 0)

    pw = np.asarray(inputs["proj_w"], f32)
    pjw = np.concatenate([pw[0:128, :], pw[128:256, :]], 1).astype(f16)

    cr = np.asarray(inputs["mix_re"], np.float64)
    ci = np.asarray(inputs["mix_im"], np.float64)
    den = np.sqrt(cr * cr + ci * ci).sum() + 1e-8
    cf2 = np.stack([cr / den, ci / den], 1).astype(f32)

    mt_m = _host_qff_matrices(inputs["qff_params"], inputs["out_w"])

    owb = np.concatenate(
        [np.asarray(inputs["out_w"], f32), np.asarray(inputs["out_b"], f32)[None, :]], 0)
    lng = np.broadcast_to(np.asarray(inputs["ln_g"], f32), (BPC, D)).copy()
    lnb = np.broadcast_to(np.asarray(inputs["ln_b"], f32), (BPC, D)).copy()
    w1 = np.asarray(inputs["cls_w1"], f32)
    cw1 = np.concatenate([w1[0:128, :], w1[128:256, :]], 1)
    cb1 = np.asarray(inputs["cls_b1"], f32)[None, :]
    w2 = np.asarray(inputs["cls_w2"], f32)
    cw2 = np.concatenate([w2[0:128, :], w2[128:256, :]], 1)
    cb2 = np.asarray(inputs["cls_b2"], f32)[None, :]
    idn = np.eye(128, dtype=f32)
    pjb = np.asarray(inputs["proj_b"], f16)[None, :]

    shared = dict(wfb=wfb, aw2=np.asarray(inputs["att_w2"], f16), ewb=ewb,
                  pjw=pjw, pjb=pjb, cf2=cf2, mt=mt_m, owb=owb, lng=lng,
                  lnb=lnb, cw1=cw1, cb1=cb1, cw2=cw2, cb2=cb2, idn=idn)

    in_maps = []
    for c in range(N_CORES):
        xc = x[c * BPC : (c + 1) * BPC]
        # xq[b, nc, cc*16+k] = x[b, cc, nc*16+k]  (c-major)
        xq_c = np.ascontiguousarray(
            xc.reshape(BPC, C_IN, NC, CH).transpose(0, 2, 1, 3).reshape(
                BPC, NC, C_IN * CH))
        m = dict(shared)
        m["xs"] = np.ascontiguousarray(xc).astype(np.float16)
        m["xq"] = xq_c
        in_maps.append(m)
    return in_maps


def kernel(**inputs):
    nc = _get_program()
    in_maps = host_prep(inputs)
    res = run_bass_kernel_spmd(nc, in_maps, core_ids=list(range(N_CORES)))
    outs = [res.results[c]["out"] for c in range(N_CORES)]
    return np.concatenate(outs, 0).astype(np.float32)


if __name__ == "__main__":
    nc = build_program()
    print("program built ok")
